# revision 1
# baseline (speedup 1.0000x reference)
"""DeepSeek-style MLA decode attention (batch=8, 128 heads, cache 512) on 8 NeuronCores.

Sharding: tensor-parallel over heads (16 heads/core).
 - q LoRA path sharded over the rank dim (Wq_down cols / Wq_up rows); partial
   q summed+scattered to head owners with a ReduceScatter.
 - Wkv_down replicated (c_kv computed fully on every core).
 - k_cache passed host-pretransposed as [h, b, d, keys]; v_cache as [h, b, keys, d].
 - o_proj input rows sharded by head; partial outputs ReduceScattered over the
   batch dim (core b returns batch b's final row).

Note: the reference's "new token" softmax is over a length-1 axis (== 1.0), so
k_new/Wk_up are dead and the new-token contribution is simply + v_new.
"""

import numpy as np

import concourse.bass as bass
import concourse.mybir as mybir
import concourse.tile as tile
from concourse import bacc
from concourse import bass_utils
from concourse.masks import make_identity

NC_ = 8                      # cores
B = 8                        # batch
H = 128                      # total heads
HP = H // NC_                # 16 heads per core
D = 128                      # head dim
L = 512                      # cache len
HID = 7168
QL = 1536
QLP = QL // NC_              # 192
KVL = 512
NH = HP * D                  # 2048 per-core head cols
SCALE = 1.0 / float(np.sqrt(D))
F32 = mybir.dt.float32
# float32r: single-pass fp32 matmul mode (1 cycle/row at N>=256 vs 4 for
# two-pass fp32). Slightly reduced multiply precision; flip off if the
# accuracy gate complains.
USE_F32R = True


F32R = mybir.dt.float32r
MMD = F32R if USE_F32R else F32  # dtype for matmul-operand tiles


def _rb(ap):
    """Bitcast a DRAM f32 source AP for DMA into a float32r tile."""
    return ap.bitcast(F32R) if USE_F32R else ap


def build_nc():
    nc = bacc.Bacc(
        "TRN2",
        target_bir_lowering=False,
        debug=False,
        enable_asserts=True,
        num_devices=NC_,
    )
    xt = nc.dram_tensor("xt", [HID, B], F32, kind="ExternalInput").ap()
    w_down = nc.dram_tensor("w_down", [HID, QLP + KVL], F32, kind="ExternalInput").ap()
    wq_up = nc.dram_tensor("wq_up", [QLP, H * D], F32, kind="ExternalInput").ap()
    wv_up = nc.dram_tensor("wv_up", [KVL, NH], F32, kind="ExternalInput").ap()
    kt = nc.dram_tensor("kt", [32, 128, 2048], F32, kind="ExternalInput").ap()
    v = nc.dram_tensor("v", [32, 128, 2048], F32, kind="ExternalInput").ap()
    wo = nc.dram_tensor("wo", [NH, HID], F32, kind="ExternalInput").ap()
    o = nc.dram_tensor("o", [1, HID], F32, kind="ExternalOutput").ap()

    rg = [list(range(NC_))]

    with tile.TileContext(nc) as tc:
        with (
            tc.tile_pool(name="const", bufs=1) as constp,
            tc.tile_pool(name="sbuf", bufs=1) as sb,
            tc.tile_pool(name="stage", bufs=2) as stg,
            tc.tile_pool(name="wdown", bufs=3) as wdp,
            tc.tile_pool(name="wqup", bufs=2) as wqp,
            tc.tile_pool(name="ktp", bufs=3) as ktp,
            tc.tile_pool(name="vp", bufs=3) as vp,
            tc.tile_pool(name="wop", bufs=3) as wop,
            tc.tile_pool(name="psbank", bufs=6, space="PSUM") as psbank,
            tc.tile_pool(name="pstr", bufs=2, space="PSUM") as pstr,
            tc.tile_pool(name="dram", bufs=1, space="DRAM") as dram,
        ):
            ident = constp.tile([128, 128], F32)
            make_identity(nc, ident[:])
            id8 = ident[0:8, 0:8]
            # uint8 one-hot columns for CopyPredicated masks (must be int dtype)
            identu8 = constp.tile([128, 128], mybir.dt.uint8, tag="identu8")
            nc.vector.tensor_copy(identu8[:], ident[:])

            # ---------------- q path: cdown = x @ [Wq_down_c | Wkv_down] ----------------
            xt_sb = constp.tile([128, 56 * B], MMD, tag="xt")
            nc.sync.dma_start(
                out=xt_sb[:].rearrange("p (c b) -> p c b", c=56),
                in_=_rb(xt).rearrange("(c p) b -> p c b", p=128),
            )
            ps_cd0 = psbank.tile([8, 512], F32, tag="bank")
            ps_cd1 = psbank.tile([8, 512], F32, tag="bank")
            for i in range(56):
                wd_t = wdp.tile([128, QLP + KVL], MMD, tag="wd")
                nc.sync.dma_start(out=wd_t[:], in_=_rb(w_down)[i * 128:(i + 1) * 128, :])
                lhs = xt_sb[:, i * B:(i + 1) * B]
                nc.tensor.matmul(
                    ps_cd0[:8, 0:512], (lhs), (wd_t[:, 0:512]),
                    start=(i == 0), stop=(i == 55),
                )
                nc.tensor.matmul(
                    ps_cd1[:8, 0:192], lhs, wd_t[:, 512:704],
                    start=(i == 0), stop=(i == 55),
                )
            cdown = sb.tile([8, QLP + KVL], F32, tag="cdown")
            nc.vector.tensor_copy(cdown[:, 0:512], ps_cd0[:8, 0:512])
            nc.vector.tensor_copy(cdown[:, 512:704], ps_cd1[:8, 0:192])

            # transposes: cqT [192, 8] (2 chunks), ckvT [512dims -> 4 chunks of [128, 8]]
            ps_cqT = pstr.tile([128, 128], F32, tag="tr")
            nc.tensor.transpose(ps_cqT[0:128, 0:8], cdown[:, 0:128], id8)
            nc.tensor.transpose(ps_cqT[0:64, 8:16], cdown[:, 128:192], id8)
            ps_ckvT = pstr.tile([128, 128], F32, tag="tr")
            for j in range(4):
                nc.tensor.transpose(
                    ps_ckvT[0:128, j * 8:(j + 1) * 8],
                    cdown[:, QLP + j * 128:QLP + (j + 1) * 128],
                    id8,
                )
            cqT = sb.tile([128, 16], MMD, tag="cqT")
            nc.vector.tensor_copy(cqT[:, 0:8], ps_cqT[:, 0:8])
            nc.vector.tensor_copy(cqT[0:64, 8:16], ps_cqT[0:64, 8:16])
            ckvT = sb.tile([128, 32], MMD, tag="ckvT")
            nc.vector.tensor_copy(ckvT[:, 0:32], ps_ckvT[:, 0:32])

            # ---------------- q_part = cq @ Wq_up_c  (8, 16384) ----------------
            # The 8 col-groups of 2048 are exactly the per-core head groups g;
            # store each to q_bounce[g] for the ReduceScatter.
            q_bounce = dram.tile([NC_ * B, NH], F32, tag="qb")
            for g in range(NC_):
                wqA = wqp.tile([128, 2048], MMD, tag="wqA")
                nc.sync.dma_start(
                    out=wqA[:], in_=_rb(wq_up)[0:128, g * 2048:(g + 1) * 2048]
                )
                wqB = wqp.tile([64, 2048], MMD, tag="wqB")
                nc.sync.dma_start(
                    out=wqB[:], in_=_rb(wq_up)[128:192, g * 2048:(g + 1) * 2048]
                )
                qstage = stg.tile([8, NH], F32, tag="qstage")
                for j in range(4):
                    ps_q = psbank.tile([8, 512], F32, tag="bank")
                    nc.tensor.matmul(
                        ps_q[:8, :], (cqT[:, 0:8]), (wqA[:, j * 512:(j + 1) * 512]),
                        start=True, stop=False,
                    )
                    nc.tensor.matmul(
                        ps_q[:8, :], (cqT[0:64, 8:16]), (wqB[:, j * 512:(j + 1) * 512]),
                        start=False, stop=True,
                    )
                    nc.vector.tensor_copy(
                        qstage[:, j * 512:(j + 1) * 512], ps_q[:8, :]
                    )
                nc.sync.dma_start(
                    out=q_bounce[g * B:(g + 1) * B, :], in_=qstage[:]
                )
            q_rs = dram.tile([B, NH], F32, tag="qrs")
            nc.gpsimd.collective_compute(
                "ReduceScatter",
                mybir.AluOpType.add,
                replica_groups=rg,
                ins=[q_bounce.opt()],
                outs=[q_rs.opt()],
            )
            qown = sb.tile([8, NH], F32, tag="qown")
            nc.sync.dma_start(out=qown[:], in_=q_rs[:])

            # ---------------- v_new = ckv @ Wv_up_c  (8, 2048) ----------------
            wvup = constp.tile([128, 4 * NH], MMD, tag="wvup")
            nc.sync.dma_start(
                out=wvup[:].rearrange("p (c n) -> p c n", c=4),
                in_=_rb(wv_up).rearrange("(c p) n -> p c n", p=128),
            )
            vnew = sb.tile([8, NH], F32, tag="vnew")
            for j in range(4):
                ps_v = psbank.tile([8, 512], F32, tag="bank")
                for cc in range(4):
                    nc.tensor.matmul(
                        ps_v[:8, :],
                        (ckvT[:, cc * 8:(cc + 1) * 8]),
                        (wvup[:, cc * NH + j * 512:cc * NH + (j + 1) * 512]),
                        start=(cc == 0), stop=(cc == 3),
                    )
                nc.vector.tensor_copy(vnew[:, j * 512:(j + 1) * 512], ps_v[:8, :])

            # qT [128 d, hb] via 16 transposes
            ps_qT = pstr.tile([128, 128], F32, tag="tr")
            for h in range(HP):
                nc.tensor.transpose(
                    ps_qT[0:128, h * 8:(h + 1) * 8],
                    qown[:, h * D:(h + 1) * D],
                    id8,
                )
            qT = sb.tile([128, 128], MMD, tag="qT")
            nc.vector.tensor_copy(qT[:], ps_qT[:])

            # ---------------- phase A: scores over k cache ----------------
            # lhsT = qT (all 128 hb) stationary; rhs = kT tile (moving, N=512).
            # Out row hb of each full-bank product is the valid score row;
            # extract it with a partition-aligned copy.
            scores = sb.tile([128, 512], F32, tag="scores")
            for t in range(32):
                kt_t = ktp.tile([128, 2048], MMD, tag="kt")
                nc.sync.dma_start(out=kt_t[:], in_=_rb(kt)[t])
                for u in range(4):
                    hb = 4 * t + u
                    ps_s = psbank.tile([128, 512], F32, tag="bank")
                    nc.tensor.matmul(
                        ps_s[:],
                        (qT[:]),
                        (kt_t[:, u * 512:(u + 1) * 512]),
                        start=True, stop=True,
                    )
                    # write only row hb (engines can't address partition hb
                    # directly: start partition must be 0/32/64/96)
                    nc.vector.copy_predicated(
                        scores[:],
                        identu8[:, hb:hb + 1].broadcast_to((128, 512)),
                        ps_s[:],
                    )

            probs = sb.tile([128, 512], F32, tag="probs")
            denom = sb.tile([128, 1], F32, tag="denom")
            nc.scalar.activation(
                probs[:], scores[:], mybir.ActivationFunctionType.Exp,
                scale=SCALE, accum_out=denom[:],
            )
            recip = sb.tile([128, 1], F32, tag="recip")
            nc.vector.reciprocal(recip[:], denom[:])
            probsn = sb.tile([128, 512], F32, tag="probsn")
            nc.vector.tensor_scalar_mul(probsn[:], probs[:], recip[:])

            ps_pT = psbank.tile([128, 512], F32, tag="bank")
            for cc in range(4):
                nc.tensor.transpose(
                    ps_pT[:, cc * 128:(cc + 1) * 128],
                    probsn[:, cc * 128:(cc + 1) * 128],
                    ident[:],
                )
            probsT = sb.tile([128, 512], MMD, tag="probsT")
            nc.vector.tensor_copy(probsT[:], ps_pT[:])

            # ---------------- phase B: attn rows = probs @ V ----------------
            # Per group of 4 hb: lhsT = probsT chunk c (all hb), rhs packs the
            # 4 hb's V chunk c side by side; accumulate over c, then extract
            # row 4g+u from column block u.
            attn = sb.tile([128, 128], F32, tag="attn")
            for g in range(32):
                v_t = vp.tile([128, 2048], MMD, tag="v")
                nc.sync.dma_start(out=v_t[:], in_=_rb(v)[g])
                ps_a = psbank.tile([128, 512], F32, tag="bank")
                for cc in range(4):
                    nc.tensor.matmul(
                        ps_a[:],
                        (probsT[:, cc * 128:(cc + 1) * 128]),
                        (v_t[:, cc * 512:(cc + 1) * 512]),
                        start=(cc == 0), stop=(cc == 3),
                    )
                for u in range(4):
                    hb = 4 * g + u
                    nc.vector.copy_predicated(
                        attn[:],
                        identu8[:, hb:hb + 1].broadcast_to((128, 128)),
                        ps_a[:, u * 128:(u + 1) * 128],
                    )

            # attnT = attn^T + v_new^T
            ps_vT = pstr.tile([128, 128], F32, tag="tr")
            for h in range(HP):
                nc.tensor.transpose(
                    ps_vT[0:128, h * 8:(h + 1) * 8],
                    vnew[:, h * D:(h + 1) * D],
                    id8,
                )
            vnewT = sb.tile([128, 128], F32, tag="vnewT")
            nc.vector.tensor_copy(vnewT[:], ps_vT[:])
            ps_aT = pstr.tile([128, 128], F32, tag="tr")
            nc.tensor.transpose(ps_aT[:], attn[:], ident[:])
            attnT = sb.tile([128, 128], MMD, tag="attnT")
            nc.vector.tensor_add(attnT[:], ps_aT[:], vnewT[:])

            # ---------------- phase C: o_part = attn^T @ Wo_c ----------------
            # Rounds of up to 6 n-chunks so the accumulators fit in the bank
            # pool; Wo streams as per-head row blocks (large contiguous runs).
            o_bounce = dram.tile([B, HID], F32, tag="ob")
            for n0, n1 in ((0, 6), (6, 12), (12, 14)):
                nn = n1 - n0
                ps_os = [
                    psbank.tile([8, 512], F32, tag="bank", name=f"ps_o{n0}_{i}")
                    for i in range(nn)
                ]
                for h in range(HP):
                    wo_t = wop.tile([128, 3072], MMD, tag="wo")
                    nc.sync.dma_start(
                        out=wo_t[:, 0:nn * 512],
                        in_=_rb(wo)[h * D:(h + 1) * D, n0 * 512:n1 * 512],
                    )
                    for i in range(nn):
                        nc.tensor.matmul(
                            ps_os[i][:8, :],
                            (attnT[:, h * 8:(h + 1) * 8]),
                            (wo_t[:, i * 512:(i + 1) * 512]),
                            start=(h == 0), stop=(h == HP - 1),
                        )
                for i in range(nn):
                    ostage = stg.tile([8, 512], F32, tag="ostage")
                    nc.vector.tensor_copy(ostage[:], ps_os[i][:8, :])
                    nc.sync.dma_start(
                        out=o_bounce[:, (n0 + i) * 512:(n0 + i + 1) * 512],
                        in_=ostage[:],
                    )

            o_rs = dram.tile([1, HID], F32, tag="ors")
            nc.gpsimd.collective_compute(
                "ReduceScatter",
                mybir.AluOpType.add,
                replica_groups=rg,
                ins=[o_bounce.opt()],
                outs=[o_rs.opt()],
            )
            nc.sync.dma_start(out=o[:], in_=o_rs[:])

    nc.compile()
    return nc


_NC_CACHE = None


def _get_nc():
    global _NC_CACHE
    if _NC_CACHE is None:
        _NC_CACHE = build_nc()
    return _NC_CACHE


def make_in_maps(x, k_cache, v_cache, Wq_down, Wq_up, Wkv_down, Wv_up, Wo):
    x2 = np.ascontiguousarray(np.asarray(x, dtype=np.float32).reshape(B, HID).T)
    in_maps = []
    for c in range(NC_):
        hs = slice(c * HP, (c + 1) * HP)
        w_down_c = np.ascontiguousarray(
            np.concatenate(
                [Wq_down[:, c * QLP:(c + 1) * QLP], Wkv_down], axis=1
            ).astype(np.float32)
        )
        wq_up_c = np.ascontiguousarray(Wq_up[c * QLP:(c + 1) * QLP, :], dtype=np.float32)
        wv_up_c = np.ascontiguousarray(
            Wv_up[:, c * HP * D:(c + 1) * HP * D], dtype=np.float32
        )
        wo_c = np.ascontiguousarray(
            Wo[c * HP * D:(c + 1) * HP * D, :], dtype=np.float32
        )
        # kt tile g holds hb=4g..4g+4 as [128 d, (t, k)]; hb=(h, b) row-major
        kt_c = np.ascontiguousarray(
            np.asarray(k_cache, dtype=np.float32)[:, hs]
            .transpose(1, 0, 3, 2)          # (16, 8, 128, 512) [h, b, d, k]
            .reshape(32, 4, 128, 512)       # [g, t, d, k]
            .transpose(0, 2, 1, 3)          # [g, d, t, k]
            .reshape(32, 128, 2048)
        )
        # v tile g holds hb=4g..4g+4 as [128 k, (c, t, d)]
        v_c = np.ascontiguousarray(
            np.asarray(v_cache, dtype=np.float32)[:, hs]
            .transpose(1, 0, 2, 3)          # (16, 8, 512, 128) [h, b, l, d]
            .reshape(32, 4, 4, 128, 128)    # [g, t, c, k, d]
            .transpose(0, 3, 2, 1, 4)       # [g, k, c, t, d]
            .reshape(32, 128, 2048)
        )
        in_maps.append(
            {
                "xt": x2,
                "w_down": w_down_c,
                "wq_up": wq_up_c,
                "wv_up": wv_up_c,
                "kt": kt_c,
                "v": v_c,
                "wo": wo_c,
            }
        )
    return in_maps


def kernel(x, k_cache, v_cache, Wq_down, Wq_up, Wkv_down, Wk_up, Wv_up, Wo, **_):
    x = np.asarray(x, dtype=np.float32)
    in_maps = make_in_maps(
        x, np.asarray(k_cache), np.asarray(v_cache),
        np.asarray(Wq_down, dtype=np.float32), np.asarray(Wq_up, dtype=np.float32),
        np.asarray(Wkv_down, dtype=np.float32), np.asarray(Wv_up, dtype=np.float32),
        np.asarray(Wo, dtype=np.float32),
    )
    nc = _get_nc()
    res = bass_utils.run_bass_kernel_spmd(nc, in_maps, core_ids=list(range(NC_)))
    out = np.stack([res.results[b]["o"] for b in range(B)], axis=0)  # (8, 1, 7168)
    return np.ascontiguousarray(out, dtype=np.float32)



# revision 16
# speedup vs baseline: 1.7783x; 1.7783x over previous
"""DeepSeek-style MLA decode attention (batch=8, 128 heads, cache 512) on 8 NeuronCores.

Sharding: tensor-parallel over heads (16 heads/core), fp16 on-device.
 - q LoRA down-proj sharded over rank cols (exact per-core cq); Wkv_down sharded
   over input rows (partial c_kv). One small AllGather ships transposed
   cq/ckv-partials to every core early; each core then computes q for its own
   heads with a column shard of Wq_up (no big mid-kernel collective).
 - k_cache host-pretransposed to [super, d, keys]; v_cache to [super, k, (c,t,d)].
 - o_proj input rows sharded by head; partial outputs ReduceScattered over the
   batch dim in 3 column chunks (overlapped with o_proj compute); core b
   returns batch b's final row.

Note: the reference's "new token" softmax is over a length-1 axis (== 1.0), so
k_new/Wk_up are dead and the new-token contribution is simply + v_new.
"""

import os

import numpy as np

import concourse.bass as bass
import concourse.mybir as mybir
import concourse.tile as tile
from concourse import bacc
from concourse import bass_utils
from concourse.masks import make_identity

# Bisect flags (1 = v2 risky variant, 0 = baseline-safe variant)
F_AG16 = int(os.environ.get("BIS_AG16", "0"))      # f16 AllGather vs f32
F_CHUNKRS = int(os.environ.get("BIS_CHUNKRS", "0"))  # 3 chunked RS vs 1
F_K64 = int(os.environ.get("BIS_K64", "0"))        # base-64 K=64 q_up MMs
F_PPRED = int(os.environ.get("BIS_PPRED", "0"))    # partial-partition copy_predicated

NC_ = 8                      # cores
B = 8                        # batch
H = 128                      # total heads
HP = H // NC_                # 16 heads per core
D = 128                      # head dim
L = 512                      # cache len
HID = 7168
QL = 1536
QLP = QL // NC_              # 192
KVL = 512
KVRP = HID // NC_            # 896 input rows of Wkv_down per core
NH = HP * D                  # 2048 per-core head cols
SCALE = 1.0 / float(np.sqrt(D))
F32 = mybir.dt.float32
F16 = mybir.dt.float16
U8 = mybir.dt.uint8

# o_proj column rounds: (start, end) in 512-col chunks of HID
ROUNDS = ((0, 6), (6, 12), (12, 14))


def build_nc():
    nc = bacc.Bacc(
        "TRN2",
        target_bir_lowering=False,
        debug=False,
        enable_asserts=False,
        num_devices=NC_,
    )
    xt = nc.dram_tensor("xt", [128, 56 * B], F16, kind="ExternalInput").ap()
    xkv = nc.dram_tensor("xkv", [128, 7 * B], F16, kind="ExternalInput").ap()
    wqd = nc.dram_tensor("wqd", [2, 128, 28 * QLP], F16, kind="ExternalInput").ap()
    wkvd = nc.dram_tensor("wkvd", [128, 7 * KVL], F16, kind="ExternalInput").ap()
    wq = nc.dram_tensor("wq", [4, 128, 16 * 512], F16, kind="ExternalInput").ap()
    wvup = nc.dram_tensor("wvup", [2, 128, 2 * NH], F16, kind="ExternalInput").ap()
    kt = nc.dram_tensor("kt", [16, 128, 4096], F16, kind="ExternalInput").ap()
    v = nc.dram_tensor("v", [16, 128, 4096], F16, kind="ExternalInput").ap()
    wos = [
        nc.dram_tensor(
            f"wo{r}", [8, 128, 2 * (n1 - n0) * 512], F16, kind="ExternalInput"
        ).ap()
        for r, (n0, n1) in enumerate(ROUNDS)
    ]
    o = nc.dram_tensor("o", [1, HID], F32, kind="ExternalOutput").ap()

    rg = [list(range(NC_))]

    with tile.TileContext(nc) as tc:
        with (
            tc.tile_pool(name="const", bufs=1) as constp,
            tc.tile_pool(name="sbuf", bufs=1) as sb,
            tc.tile_pool(name="stage", bufs=2) as stg,
            tc.tile_pool(name="wqdp", bufs=2) as wqdp,
            tc.tile_pool(name="wqp", bufs=2) as wqp,
            tc.tile_pool(name="ktp", bufs=3) as ktp,
            tc.tile_pool(name="vp", bufs=3) as vp,
            tc.tile_pool(name="wop", bufs=3) as wop,
            tc.tile_pool(name="psbank", bufs=6, space="PSUM") as psbank,
            tc.tile_pool(name="pstr", bufs=2, space="PSUM") as pstr,
            tc.tile_pool(name="dram", bufs=1, space="DRAM") as dram,
        ):
            ident = constp.tile([128, 128], F32)
            make_identity(nc, ident[:])
            id8 = ident[0:8, 0:8]
            ident16 = constp.tile([128, 128], F16, tag="ident16")
            nc.vector.tensor_copy(ident16[:], ident[:])
            # uint8 one-hot columns for CopyPredicated masks (must be int dtype)
            identu8 = constp.tile([128, 128], U8, tag="identu8")
            nc.vector.tensor_copy(identu8[:], ident[:])

            # ---------------- LoRA down: cq = x @ Wq_down_c, ckv partial ----------------
            xt_sb = constp.tile([128, 56 * B], F16, tag="xt")
            nc.sync.dma_start(out=xt_sb[:], in_=xt)
            xkv_sb = constp.tile([128, 7 * B], F16, tag="xkv")
            nc.sync.dma_start(out=xkv_sb[:], in_=xkv)
            wkvd_sb = constp.tile([128, 7 * KVL], F16, tag="wkvd")
            nc.sync.dma_start(out=wkvd_sb[:], in_=wkvd)

            ps_cq = psbank.tile([8, 512], F32, tag="bank")
            ps_ckv = psbank.tile([8, 512], F32, tag="bank")
            wqd_ts = []
            for j in range(2):
                wqd_t = wqdp.tile([128, 28 * QLP], F16, tag="wqd")
                nc.sync.dma_start(out=wqd_t[:], in_=wqd[j])
                wqd_ts.append(wqd_t)
            for i in range(56):
                j, ii = divmod(i, 28)
                nc.tensor.matmul(
                    ps_cq[:8, 0:QLP],
                    xt_sb[:, i * B:(i + 1) * B],
                    wqd_ts[j][:, ii * QLP:(ii + 1) * QLP],
                    start=(i == 0), stop=(i == 55),
                )
            for i in range(7):
                nc.tensor.matmul(
                    ps_ckv[:8, :],
                    xkv_sb[:, i * B:(i + 1) * B],
                    wkvd_sb[:, i * KVL:(i + 1) * KVL],
                    start=(i == 0), stop=(i == 6),
                )
            cdq = sb.tile([8, QLP], F32, tag="cdq")
            nc.vector.tensor_copy(cdq[:], ps_cq[:8, 0:QLP])
            cdkv = sb.tile([8, KVL], F32, tag="cdkv")
            nc.vector.tensor_copy(cdkv[:], ps_ckv[:8, :])

            # transposes -> ag_in staging [128, 6*8] f16
            # x-cols: 0: cq rows 0-127, 1: cq rows 128-191 (parts 0-63),
            #         2-5: ckv chunks of 128
            ps_cqT = pstr.tile([128, 16], F32, tag="tr")
            nc.tensor.transpose(ps_cqT[0:128, 0:8], cdq[:, 0:128], id8)
            nc.tensor.transpose(ps_cqT[0:64, 8:16], cdq[:, 128:192], id8)
            ps_ckvT = pstr.tile([128, 32], F32, tag="tr")
            for j in range(4):
                nc.tensor.transpose(
                    ps_ckvT[0:128, j * 8:(j + 1) * 8],
                    cdkv[:, j * 128:(j + 1) * 128],
                    id8,
                )
            AGDT = F16 if F_AG16 else F32
            ag_in_sb = sb.tile([128, 48], AGDT, tag="agin")
            nc.vector.memset(ag_in_sb[:, 8:16], 0.0)
            nc.vector.tensor_copy(ag_in_sb[:, 0:8], ps_cqT[:, 0:8])
            nc.vector.tensor_copy(ag_in_sb[0:64, 8:16], ps_cqT[0:64, 8:16])
            nc.vector.tensor_copy(ag_in_sb[:, 16:48], ps_ckvT[:, 0:32])

            ag_in = dram.tile([6 * 128, B], AGDT, tag="agi")
            nc.sync.dma_start(
                out=ag_in[:].rearrange("(x p) b -> p x b", p=128),
                in_=ag_in_sb[:].rearrange("p (x b) -> p x b", x=6),
            )
            ag_out = dram.tile([NC_ * 6 * 128, B], AGDT, tag="ago")
            nc.gpsimd.collective_compute(
                "AllGather",
                mybir.AluOpType.bypass,
                replica_groups=rg,
                ins=[ag_in.opt()],
                outs=[ag_out.opt()],
            )
            cq_stage = sb.tile([128, 48 * B], AGDT, tag="cqstage")
            nc.sync.dma_start(
                out=cq_stage[:].rearrange("p (x b) -> p x b", x=48),
                in_=ag_out[:].rearrange("(x p) b -> p x b", p=128),
            )
            if F_AG16:
                cqmm = cq_stage
            else:
                cqmm = sb.tile([128, 48 * B], F16, tag="cqmm")
                nc.vector.tensor_copy(cqmm[:], cq_stage[:])

            # ---------------- q_own = cq @ Wq_up_c  (8, 2048) ----------------
            # contraction over 1536 rank dims as 24 K=64 chunks; c64 = r*3 + j64
            qstage = sb.tile([8, NH], F32, tag="qstage")
            for n in range(4):
                wq_t = wqp.tile([128, 16 * 512], F16, tag="wq")
                nc.sync.dma_start(out=wq_t[:], in_=wq[n])
                ps_q = psbank.tile([8, 512], F32, tag="bank")
                if F_K64:
                    for c64 in range(24):
                        r, j64 = divmod(c64, 3)
                        po = 64 if j64 == 1 else 0
                        xc = r * 6 + (1 if j64 == 2 else 0)
                        xw = r * 2 + (1 if j64 == 2 else 0)
                        nc.tensor.matmul(
                            ps_q[:8, :],
                            cqmm[po:po + 64, xc * 8:(xc + 1) * 8],
                            wq_t[po:po + 64, xw * 512:(xw + 1) * 512],
                            start=(c64 == 0), stop=(c64 == 23),
                        )
                else:
                    for r in range(8):
                        nc.tensor.matmul(
                            ps_q[:8, :],
                            cqmm[0:128, (r * 6) * 8:(r * 6 + 1) * 8],
                            wq_t[0:128, (2 * r) * 512:(2 * r + 1) * 512],
                            start=(r == 0), stop=False,
                        )
                        nc.tensor.matmul(
                            ps_q[:8, :],
                            cqmm[0:64, (r * 6 + 1) * 8:(r * 6 + 2) * 8],
                            wq_t[0:64, (2 * r + 1) * 512:(2 * r + 2) * 512],
                            start=False, stop=(r == 7),
                        )
                nc.vector.tensor_copy(qstage[:, n * 512:(n + 1) * 512], ps_q[:8, :])

            # ckv full = sum of the 8 gathered partials -> ckvT16 [128, 4*8] f16
            ckvT = sb.tile([128, 32], AGDT, tag="ckvT")
            nc.vector.tensor_copy(ckvT[:], cq_stage[:, 16:48])
            for r in range(1, 8):
                base = (r * 6 + 2) * 8
                nc.vector.tensor_add(ckvT[:], ckvT[:], cq_stage[:, base:base + 32])
            ckvT16 = sb.tile([128, 32], F16, tag="ckvT16")
            nc.vector.tensor_copy(ckvT16[:], ckvT[:])

            # qT [128 d, 128 hb] f16 via 16 transposes
            ps_qT = pstr.tile([128, 128], F32, tag="tr")
            for h in range(HP):
                nc.tensor.transpose(
                    ps_qT[0:128, h * 8:(h + 1) * 8],
                    qstage[:, h * D:(h + 1) * D],
                    id8,
                )
            qT = sb.tile([128, 128], F16, tag="qT")
            nc.vector.tensor_copy(qT[:], ps_qT[:])

            # ---------------- phase A: scores over k cache ----------------
            scores = sb.tile([128, 512], F32, tag="scores")
            for s in range(16):
                kt_t = ktp.tile([128, 4096], F16, tag="kt")
                nc.sync.dma_start(out=kt_t[:], in_=kt[s])
                pa = 32 * (s // 4)
                for u in range(8):
                    hb = 8 * s + u
                    ps_s = psbank.tile([128, 512], F32, tag="bank")
                    nc.tensor.matmul(
                        ps_s[:], qT[:], kt_t[:, u * 512:(u + 1) * 512],
                        start=True, stop=True,
                    )
                    if F_PPRED:
                        nc.vector.copy_predicated(
                            scores[pa:pa + 32, :],
                            identu8[pa:pa + 32, hb:hb + 1].broadcast_to((32, 512)),
                            ps_s[pa:pa + 32, :],
                        )
                    else:
                        nc.vector.copy_predicated(
                            scores[:],
                            identu8[:, hb:hb + 1].broadcast_to((128, 512)),
                            ps_s[:],
                        )

            probs = sb.tile([128, 512], F32, tag="probs")
            denom = sb.tile([128, 1], F32, tag="denom")
            nc.scalar.activation(
                probs[:], scores[:], mybir.ActivationFunctionType.Exp,
                scale=SCALE, accum_out=denom[:],
            )
            recip = sb.tile([128, 1], F32, tag="recip")
            nc.vector.reciprocal(recip[:], denom[:])
            probsn = sb.tile([128, 512], F32, tag="probsn")
            nc.vector.tensor_scalar_mul(probsn[:], probs[:], recip[:])

            probsT = sb.tile([128, 512], F16, tag="probsT")
            for cc in range(4):
                ps_pT = pstr.tile([128, 128], F32, tag="tr")
                nc.tensor.transpose(
                    ps_pT[:], probsn[:, cc * 128:(cc + 1) * 128], ident[:]
                )
                nc.vector.tensor_copy(probsT[:, cc * 128:(cc + 1) * 128], ps_pT[:])

            # ---------------- v_new = ckv @ Wv_up_c (8, 2048), off critical path ----
            wvup_ts = []
            for j in range(2):
                wv_t = wqp.tile([128, 2 * NH], F16, tag="wq", name=f"wvup{j}")
                nc.sync.dma_start(out=wv_t[:], in_=wvup[j])
                wvup_ts.append(wv_t)
            vnew = sb.tile([8, NH], F32, tag="vnew")
            for n in range(4):
                ps_v = psbank.tile([8, 512], F32, tag="bank")
                for cc in range(4):
                    nc.tensor.matmul(
                        ps_v[:8, :],
                        ckvT16[:, cc * 8:(cc + 1) * 8],
                        wvup_ts[cc // 2][:, (cc % 2) * NH + n * 512:
                                         (cc % 2) * NH + (n + 1) * 512],
                        start=(cc == 0), stop=(cc == 3),
                    )
                nc.vector.tensor_copy(vnew[:, n * 512:(n + 1) * 512], ps_v[:8, :])
            ps_vT = pstr.tile([128, 128], F32, tag="tr")
            for h in range(HP):
                nc.tensor.transpose(
                    ps_vT[0:128, h * 8:(h + 1) * 8],
                    vnew[:, h * D:(h + 1) * D],
                    id8,
                )
            vnewT = sb.tile([128, 128], F32, tag="vnewT")
            nc.vector.tensor_copy(vnewT[:], ps_vT[:])

            # ---------------- phase B: attn rows = probs @ V ----------------
            attn = sb.tile([128, 128], F32, tag="attn")
            for s in range(16):
                v_t = vp.tile([128, 4096], F16, tag="v")
                nc.sync.dma_start(out=v_t[:], in_=v[s])
                pa = 32 * (s // 4)
                for gg in range(2):
                    g = 2 * s + gg
                    ps_a = psbank.tile([128, 512], F32, tag="bank")
                    for cc in range(4):
                        nc.tensor.matmul(
                            ps_a[:],
                            probsT[:, cc * 128:(cc + 1) * 128],
                            v_t[:, gg * 2048 + cc * 512:gg * 2048 + (cc + 1) * 512],
                            start=(cc == 0), stop=(cc == 3),
                        )
                    for u in range(4):
                        hb = 4 * g + u
                        if F_PPRED:
                            nc.vector.copy_predicated(
                                attn[pa:pa + 32, :],
                                identu8[pa:pa + 32, hb:hb + 1].broadcast_to((32, 128)),
                                ps_a[pa:pa + 32, u * 128:(u + 1) * 128],
                            )
                        else:
                            nc.vector.copy_predicated(
                                attn[:],
                                identu8[:, hb:hb + 1].broadcast_to((128, 128)),
                                ps_a[:, u * 128:(u + 1) * 128],
                            )

            # attnT = attn^T + v_new^T  (f16 for o_proj)
            ps_aT = pstr.tile([128, 128], F32, tag="tr")
            nc.tensor.transpose(ps_aT[:], attn[:], ident[:])
            attnT = sb.tile([128, 128], F16, tag="attnT")
            nc.vector.tensor_add(attnT[:], ps_aT[:], vnewT[:])

            # ---------------- phase C: o_part = attn^T @ Wo_c, RS ----------
            if F_CHUNKRS:
                o_rss = []
                for r, (n0, n1) in enumerate(ROUNDS):
                    nn = n1 - n0
                    ps_os = [
                        psbank.tile([8, 512], F32, tag="bank", name=f"ps_o{r}_{i}")
                        for i in range(nn)
                    ]
                    for hp in range(8):
                        wo_t = wop.tile([128, 2 * 6 * 512], F16, tag="wo")
                        nc.sync.dma_start(out=wo_t[:, 0:2 * nn * 512], in_=wos[r][hp])
                        for i2 in range(2):
                            h = 2 * hp + i2
                            for i in range(nn):
                                nc.tensor.matmul(
                                    ps_os[i][:8, :],
                                    attnT[:, h * 8:(h + 1) * 8],
                                    wo_t[:, (i2 * nn + i) * 512:(i2 * nn + i + 1) * 512],
                                    start=(h == 0), stop=(h == 15),
                                )
                    ostage = stg.tile([8, 6 * 512], F32, tag="ostage")
                    for i in range(nn):
                        nc.vector.tensor_copy(
                            ostage[:, i * 512:(i + 1) * 512], ps_os[i][:8, :]
                        )
                    o_bounce = dram.tile([B, nn * 512], F32, tag=f"ob{r}")
                    nc.sync.dma_start(out=o_bounce[:], in_=ostage[:, 0:nn * 512])
                    o_rs = dram.tile([1, nn * 512], F32, tag=f"ors{r}")
                    nc.gpsimd.collective_compute(
                        "ReduceScatter",
                        mybir.AluOpType.add,
                        replica_groups=rg,
                        ins=[o_bounce.opt()],
                        outs=[o_rs.opt()],
                    )
                    o_rss.append((o_rs, n0, n1))
                for o_rs, n0, n1 in o_rss:
                    nc.sync.dma_start(out=o[0:1, n0 * 512:n1 * 512], in_=o_rs[:])
            else:
                o_bounce = dram.tile([B, HID], F32, tag="ob")
                for r, (n0, n1) in enumerate(ROUNDS):
                    nn = n1 - n0
                    ps_os = [
                        psbank.tile([8, 512], F32, tag="bank", name=f"ps_o{r}_{i}")
                        for i in range(nn)
                    ]
                    for hp in range(8):
                        wo_t = wop.tile([128, 2 * 6 * 512], F16, tag="wo")
                        nc.sync.dma_start(out=wo_t[:, 0:2 * nn * 512], in_=wos[r][hp])
                        for i2 in range(2):
                            h = 2 * hp + i2
                            for i in range(nn):
                                nc.tensor.matmul(
                                    ps_os[i][:8, :],
                                    attnT[:, h * 8:(h + 1) * 8],
                                    wo_t[:, (i2 * nn + i) * 512:(i2 * nn + i + 1) * 512],
                                    start=(h == 0), stop=(h == 15),
                                )
                    ostage = stg.tile([8, 6 * 512], F32, tag="ostage")
                    for i in range(nn):
                        nc.vector.tensor_copy(
                            ostage[:, i * 512:(i + 1) * 512], ps_os[i][:8, :]
                        )
                    nc.sync.dma_start(
                        out=o_bounce[:, n0 * 512:n1 * 512],
                        in_=ostage[:, 0:nn * 512],
                    )
                o_rs = dram.tile([1, HID], F32, tag="ors")
                nc.gpsimd.collective_compute(
                    "ReduceScatter",
                    mybir.AluOpType.add,
                    replica_groups=rg,
                    ins=[o_bounce.opt()],
                    outs=[o_rs.opt()],
                )
                nc.sync.dma_start(out=o[:], in_=o_rs[:])

    nc.compile()
    return nc


_NC_CACHE = None


def _get_nc():
    global _NC_CACHE
    if _NC_CACHE is None:
        _NC_CACHE = build_nc()
    return _NC_CACHE


def make_in_maps(x, k_cache, v_cache, Wq_down, Wq_up, Wkv_down, Wv_up, Wo):
    f16 = np.float16
    x2 = np.asarray(x, dtype=np.float32).reshape(B, HID).T  # [7168, 8]
    xt_t = np.ascontiguousarray(
        x2.reshape(56, 128, B).transpose(1, 0, 2).reshape(128, 56 * B).astype(f16)
    )
    Wq_down = np.asarray(Wq_down, dtype=np.float32)
    Wq_up = np.asarray(Wq_up, dtype=np.float32)
    Wkv_down = np.asarray(Wkv_down, dtype=np.float32)
    Wv_up = np.asarray(Wv_up, dtype=np.float32)
    Wo = np.asarray(Wo, dtype=np.float32)
    k_cache = np.asarray(k_cache, dtype=np.float32)
    v_cache = np.asarray(v_cache, dtype=np.float32)

    in_maps = []
    for c in range(NC_):
        hs = slice(c * HP, (c + 1) * HP)
        wqd_c = (
            Wq_down[:, c * QLP:(c + 1) * QLP]
            .reshape(2, 28, 128, QLP).transpose(0, 2, 1, 3)
            .reshape(2, 128, 28 * QLP).astype(f16)
        )
        wkvd_c = (
            Wkv_down[c * KVRP:(c + 1) * KVRP, :]
            .reshape(7, 128, KVL).transpose(1, 0, 2).reshape(128, 7 * KVL)
            .astype(f16)
        )
        xkv_c = np.ascontiguousarray(
            x2.reshape(56, 128, B)[7 * c:7 * c + 7]
            .transpose(1, 0, 2).reshape(128, 7 * B).astype(f16)
        )
        wq_shard = Wq_up[:, c * NH:(c + 1) * NH]
        # pad rows to 256 per rank (rows r*256+192..r*256+255 zero) so rhs
        # partition offsets match the gathered cq layout
        wq_pad = np.zeros((2048, NH), np.float32)
        for r in range(8):
            wq_pad[r * 256:r * 256 + QLP] = wq_shard[r * QLP:(r + 1) * QLP]
        wq_c = np.stack([
            wq_pad[:, n * 512:(n + 1) * 512]
            .reshape(16, 128, 512).transpose(1, 0, 2).reshape(128, 16 * 512)
            for n in range(4)
        ]).astype(f16)
        # wvup: [512, 2048] -> 2 chunks of cc-pairs [128, 2*2048]
        wvup_c = (
            Wv_up[:, c * NH:(c + 1) * NH]
            .reshape(2, 2, 128, NH).transpose(0, 2, 1, 3).reshape(2, 128, 2 * NH)
            .astype(f16)
        )
        kt_c = (
            k_cache[:, hs]
            .transpose(1, 0, 3, 2)          # (16, 8, 128, 512) [h, b, d, k]
            .reshape(32, 4, 128, 512)       # [g, t, d, k]
            .transpose(0, 2, 1, 3)          # [g, d, t, k]
            .reshape(16, 2, 128, 2048)      # [s, g2, d, tk]
            .transpose(0, 2, 1, 3)
            .reshape(16, 128, 4096)
            .astype(f16)
        )
        v_c = (
            v_cache[:, hs]
            .transpose(1, 0, 2, 3)          # (16, 8, 512, 128) [h, b, l, d]
            .reshape(32, 4, 4, 128, 128)    # [g, t, c, k, d]
            .transpose(0, 3, 2, 1, 4)       # [g, k, c, t, d]
            .reshape(16, 2, 128, 2048)
            .transpose(0, 2, 1, 3)
            .reshape(16, 128, 4096)
            .astype(f16)
        )
        wo_shard = Wo[c * NH:(c + 1) * NH, :]  # [2048, 7168]
        wo_cs = []
        for (n0, n1) in ROUNDS:
            nn = n1 - n0
            wo_r = (
                wo_shard[:, n0 * 512:n1 * 512]
                .reshape(8, 2, 128, nn * 512).transpose(0, 2, 1, 3)
                .reshape(8, 128, 2 * nn * 512).astype(f16)
            )
            wo_cs.append(wo_r)
        in_maps.append(
            {
                "xt": xt_t,
                "xkv": xkv_c,
                "wqd": np.ascontiguousarray(wqd_c),
                "wkvd": np.ascontiguousarray(wkvd_c),
                "wq": np.ascontiguousarray(wq_c),
                "wvup": np.ascontiguousarray(wvup_c),
                "kt": np.ascontiguousarray(kt_c),
                "v": np.ascontiguousarray(v_c),
                "wo0": np.ascontiguousarray(wo_cs[0]),
                "wo1": np.ascontiguousarray(wo_cs[1]),
                "wo2": np.ascontiguousarray(wo_cs[2]),
            }
        )
    return in_maps


def kernel(x, k_cache, v_cache, Wq_down, Wq_up, Wkv_down, Wk_up, Wv_up, Wo, **_):
    in_maps = make_in_maps(
        np.asarray(x), np.asarray(k_cache), np.asarray(v_cache),
        np.asarray(Wq_down), np.asarray(Wq_up),
        np.asarray(Wkv_down), np.asarray(Wv_up), np.asarray(Wo),
    )
    nc = _get_nc()
    res = bass_utils.run_bass_kernel_spmd(nc, in_maps, core_ids=list(range(NC_)))
    out = np.stack([res.results[b]["o"] for b in range(B)], axis=0)  # (8, 1, 7168)
    return np.ascontiguousarray(out, dtype=np.float32)


# revision 20
# speedup vs baseline: 1.8092x; 1.0174x over previous
"""DeepSeek-style MLA decode attention (batch=8, 128 heads, cache 512) on 8 NeuronCores.

Sharding: tensor-parallel over heads (16 heads/core), fp16 on-device.
 - q LoRA down-proj sharded over rank cols (exact per-core cq); Wkv_down sharded
   over input rows (partial c_kv). One small AllGather ships transposed
   cq/ckv-partials to every core early; each core then computes q for its own
   heads with a column shard of Wq_up (no big mid-kernel collective).
 - k_cache host-pretransposed to [super, d, keys]; v_cache to [super, k, (c,t,d)].
 - o_proj input rows sharded by head; partial outputs ReduceScattered over the
   batch dim in 3 column chunks (overlapped with o_proj compute); core b
   returns batch b's final row.

Note: the reference's "new token" softmax is over a length-1 axis (== 1.0), so
k_new/Wk_up are dead and the new-token contribution is simply + v_new.
"""

import os

import numpy as np

import concourse.bass as bass
import concourse.mybir as mybir
import concourse.tile as tile
from concourse import bacc
from concourse import bass_utils
from concourse.masks import make_identity

# Bisect flags (1 = v2 risky variant, 0 = baseline-safe variant)
F_AG16 = int(os.environ.get("BIS_AG16", "0"))      # f16 AllGather vs f32
F_CHUNKRS = int(os.environ.get("BIS_CHUNKRS", "1"))  # 3 chunked RS vs 1
F_K64 = int(os.environ.get("BIS_K64", "0"))        # base-64 K=64 q_up MMs
F_PPRED = int(os.environ.get("BIS_PPRED", "1"))    # partial-partition copy_predicated

NC_ = 8                      # cores
B = 8                        # batch
H = 128                      # total heads
HP = H // NC_                # 16 heads per core
D = 128                      # head dim
L = 512                      # cache len
HID = 7168
QL = 1536
QLP = QL // NC_              # 192
KVL = 512
KVRP = HID // NC_            # 896 input rows of Wkv_down per core
NH = HP * D                  # 2048 per-core head cols
SCALE = 1.0 / float(np.sqrt(D))
F32 = mybir.dt.float32
F16 = mybir.dt.float16
U8 = mybir.dt.uint8

# o_proj column rounds: (start, end) in 512-col chunks of HID
ROUNDS = ((0, 6), (6, 12), (12, 14))


def build_nc():
    nc = bacc.Bacc(
        "TRN2",
        target_bir_lowering=False,
        debug=False,
        enable_asserts=False,
        num_devices=NC_,
    )
    xt = nc.dram_tensor("xt", [128, 56 * B], F16, kind="ExternalInput").ap()
    xkv = nc.dram_tensor("xkv", [128, 7 * B], F16, kind="ExternalInput").ap()
    wqd = nc.dram_tensor("wqd", [2, 128, 28 * QLP], F16, kind="ExternalInput").ap()
    wkvd = nc.dram_tensor("wkvd", [128, 7 * KVL], F16, kind="ExternalInput").ap()
    wq = nc.dram_tensor("wq", [4, 128, 16 * 512], F16, kind="ExternalInput").ap()
    wvup = nc.dram_tensor("wvup", [2, 128, 2 * NH], F16, kind="ExternalInput").ap()
    kt = nc.dram_tensor("kt", [16, 128, 4096], F16, kind="ExternalInput").ap()
    v = nc.dram_tensor("v", [16, 128, 4096], F16, kind="ExternalInput").ap()
    wos = [
        nc.dram_tensor(
            f"wo{r}", [8, 128, 2 * (n1 - n0) * 512], F16, kind="ExternalInput"
        ).ap()
        for r, (n0, n1) in enumerate(ROUNDS)
    ]
    o = nc.dram_tensor("o", [1, HID], F32, kind="ExternalOutput").ap()

    rg = [list(range(NC_))]

    with tile.TileContext(nc) as tc:
        with (
            tc.tile_pool(name="const", bufs=1) as constp,
            tc.tile_pool(name="sbuf", bufs=1) as sb,
            tc.tile_pool(name="stage", bufs=2) as stg,
            tc.tile_pool(name="wqdp", bufs=1) as wqdp,
            tc.tile_pool(name="wqp", bufs=2) as wqp,
            tc.tile_pool(name="ktp", bufs=6) as ktp,
            tc.tile_pool(name="vp", bufs=3) as vp,
            tc.tile_pool(name="psbank", bufs=6, space="PSUM") as psbank,
            tc.tile_pool(name="pstr", bufs=2, space="PSUM") as pstr,
            tc.tile_pool(name="dram", bufs=1, space="DRAM") as dram,
        ):
            ident = constp.tile([128, 128], F32)
            make_identity(nc, ident[:])
            id8 = ident[0:8, 0:8]
            # uint8 one-hot columns for CopyPredicated masks (must be int dtype)
            identu8 = constp.tile([128, 128], U8, tag="identu8")
            nc.vector.tensor_copy(identu8[:], ident[:])

            # ---------------- LoRA down: cq = x @ Wq_down_c, ckv partial ----------------
            xt_sb = constp.tile([128, 56 * B], F16, tag="xt")
            nc.sync.dma_start(out=xt_sb[:], in_=xt)
            xkv_sb = constp.tile([128, 7 * B], F16, tag="xkv")
            nc.sync.dma_start(out=xkv_sb[:], in_=xkv)
            wkvd_sb = constp.tile([128, 7 * KVL], F16, tag="wkvd")
            nc.sync.dma_start(out=wkvd_sb[:], in_=wkvd)

            ps_cq = psbank.tile([8, 512], F32, tag="bank")
            ps_ckv = psbank.tile([8, 512], F32, tag="bank")
            wqd_ts = []
            for j in range(2):
                wqd_t = wqdp.tile([128, 28 * QLP], F16, tag="wqd")
                nc.sync.dma_start(out=wqd_t[:], in_=wqd[j])
                wqd_ts.append(wqd_t)
            for i in range(56):
                j, ii = divmod(i, 28)
                nc.tensor.matmul(
                    ps_cq[:8, 0:QLP],
                    xt_sb[:, i * B:(i + 1) * B],
                    wqd_ts[j][:, ii * QLP:(ii + 1) * QLP],
                    start=(i == 0), stop=(i == 55),
                )
            for i in range(7):
                nc.tensor.matmul(
                    ps_ckv[:8, :],
                    xkv_sb[:, i * B:(i + 1) * B],
                    wkvd_sb[:, i * KVL:(i + 1) * KVL],
                    start=(i == 0), stop=(i == 6),
                )
            cdq = sb.tile([8, QLP], F32, tag="cdq")
            nc.vector.tensor_copy(cdq[:], ps_cq[:8, 0:QLP])
            cdkv = sb.tile([8, KVL], F32, tag="cdkv")
            nc.vector.tensor_copy(cdkv[:], ps_ckv[:8, :])

            # transposes -> ag_in staging [128, 6*8] f16
            # x-cols: 0: cq rows 0-127, 1: cq rows 128-191 (parts 0-63),
            #         2-5: ckv chunks of 128
            ps_cqT = pstr.tile([128, 16], F32, tag="tr")
            nc.tensor.transpose(ps_cqT[0:128, 0:8], cdq[:, 0:128], id8)
            nc.tensor.transpose(ps_cqT[0:64, 8:16], cdq[:, 128:192], id8)
            ps_ckvT = pstr.tile([128, 32], F32, tag="tr")
            for j in range(4):
                nc.tensor.transpose(
                    ps_ckvT[0:128, j * 8:(j + 1) * 8],
                    cdkv[:, j * 128:(j + 1) * 128],
                    id8,
                )
            AGDT = F16 if F_AG16 else F32
            ag_in_sb = sb.tile([128, 48], AGDT, tag="agin")
            nc.vector.memset(ag_in_sb[:, 8:16], 0.0)
            nc.vector.tensor_copy(ag_in_sb[:, 0:8], ps_cqT[:, 0:8])
            nc.vector.tensor_copy(ag_in_sb[0:64, 8:16], ps_cqT[0:64, 8:16])
            nc.vector.tensor_copy(ag_in_sb[:, 16:48], ps_ckvT[:, 0:32])

            ag_in = dram.tile([6 * 128, B], AGDT, tag="agi")
            nc.sync.dma_start(
                out=ag_in[:].rearrange("(x p) b -> p x b", p=128),
                in_=ag_in_sb[:].rearrange("p (x b) -> p x b", x=6),
            )
            ag_out = dram.tile([NC_ * 6 * 128, B], AGDT, tag="ago")
            nc.gpsimd.collective_compute(
                "AllGather",
                mybir.AluOpType.bypass,
                replica_groups=rg,
                ins=[ag_in.opt()],
                outs=[ag_out.opt()],
            )
            cq_stage = sb.tile([128, 48 * B], AGDT, tag="cqstage")
            nc.sync.dma_start(
                out=cq_stage[:].rearrange("p (x b) -> p x b", x=48),
                in_=ag_out[:].rearrange("(x p) b -> p x b", p=128),
            )
            if F_AG16:
                cqmm = cq_stage
            else:
                cqmm = sb.tile([128, 48 * B], F16, tag="cqmm")
                nc.vector.tensor_copy(cqmm[:], cq_stage[:])

            # ---------------- q_own = cq @ Wq_up_c  (8, 2048) ----------------
            # contraction over 1536 rank dims as 24 K=64 chunks; c64 = r*3 + j64
            qstage = sb.tile([8, NH], F32, tag="qstage")
            for n in range(4):
                wq_t = wqp.tile([128, 16 * 512], F16, tag="wq")
                nc.sync.dma_start(out=wq_t[:], in_=wq[n])
                ps_q = psbank.tile([8, 512], F32, tag="bank")
                if F_K64:
                    for c64 in range(24):
                        r, j64 = divmod(c64, 3)
                        po = 64 if j64 == 1 else 0
                        xc = r * 6 + (1 if j64 == 2 else 0)
                        xw = r * 2 + (1 if j64 == 2 else 0)
                        nc.tensor.matmul(
                            ps_q[:8, :],
                            cqmm[po:po + 64, xc * 8:(xc + 1) * 8],
                            wq_t[po:po + 64, xw * 512:(xw + 1) * 512],
                            start=(c64 == 0), stop=(c64 == 23),
                        )
                else:
                    for r in range(8):
                        nc.tensor.matmul(
                            ps_q[:8, :],
                            cqmm[0:128, (r * 6) * 8:(r * 6 + 1) * 8],
                            wq_t[0:128, (2 * r) * 512:(2 * r + 1) * 512],
                            start=(r == 0), stop=False,
                        )
                        nc.tensor.matmul(
                            ps_q[:8, :],
                            cqmm[0:64, (r * 6 + 1) * 8:(r * 6 + 2) * 8],
                            wq_t[0:64, (2 * r + 1) * 512:(2 * r + 2) * 512],
                            start=False, stop=(r == 7),
                        )
                nc.vector.tensor_copy(qstage[:, n * 512:(n + 1) * 512], ps_q[:8, :])

            # ckv full = sum of the 8 gathered partials -> ckvT16 [128, 4*8] f16
            ckvT = sb.tile([128, 32], AGDT, tag="ckvT")
            nc.vector.tensor_copy(ckvT[:], cq_stage[:, 16:48])
            for r in range(1, 8):
                base = (r * 6 + 2) * 8
                nc.vector.tensor_add(ckvT[:], ckvT[:], cq_stage[:, base:base + 32])
            ckvT16 = sb.tile([128, 32], F16, tag="ckvT16")
            nc.vector.tensor_copy(ckvT16[:], ckvT[:])

            # qT [128 d, 128 hb] f16 via 16 transposes
            ps_qT = pstr.tile([128, 128], F32, tag="tr")
            for h in range(HP):
                nc.tensor.transpose(
                    ps_qT[0:128, h * 8:(h + 1) * 8],
                    qstage[:, h * D:(h + 1) * D],
                    id8,
                )
            qT = sb.tile([128, 128], F16, tag="qT")
            nc.vector.tensor_copy(qT[:], ps_qT[:])

            # ---------------- phase A: scores over k cache ----------------
            scores = sb.tile([128, 512], F32, tag="scores")
            for s in range(16):
                kt_t = ktp.tile([128, 4096], F16, tag="kt")
                nc.sync.dma_start(out=kt_t[:], in_=kt[s])
                pa = 32 * (s // 4)
                for u in range(8):
                    hb = 8 * s + u
                    ps_s = psbank.tile([128, 512], F32, tag="bank")
                    nc.tensor.matmul(
                        ps_s[:], qT[:], kt_t[:, u * 512:(u + 1) * 512],
                        start=True, stop=True,
                    )
                    if F_PPRED:
                        nc.vector.copy_predicated(
                            scores[pa:pa + 32, :],
                            identu8[pa:pa + 32, hb:hb + 1].broadcast_to((32, 512)),
                            ps_s[pa:pa + 32, :],
                        )
                    else:
                        nc.vector.copy_predicated(
                            scores[:],
                            identu8[:, hb:hb + 1].broadcast_to((128, 512)),
                            ps_s[:],
                        )

            probs = sb.tile([128, 512], F32, tag="probs")
            denom = sb.tile([128, 1], F32, tag="denom")
            nc.scalar.activation(
                probs[:], scores[:], mybir.ActivationFunctionType.Exp,
                scale=SCALE, accum_out=denom[:],
            )
            recip = sb.tile([128, 1], F32, tag="recip")
            nc.vector.reciprocal(recip[:], denom[:])
            probsn = sb.tile([128, 512], F32, tag="probsn")
            nc.vector.tensor_scalar_mul(probsn[:], probs[:], recip[:])

            probsT = sb.tile([128, 512], F16, tag="probsT")
            for cc in range(4):
                ps_pT = pstr.tile([128, 128], F32, tag="tr")
                nc.tensor.transpose(
                    ps_pT[:], probsn[:, cc * 128:(cc + 1) * 128], ident[:]
                )
                nc.vector.tensor_copy(probsT[:, cc * 128:(cc + 1) * 128], ps_pT[:])

            # ---------------- v_new = ckv @ Wv_up_c (8, 2048), off critical path ----
            wvup_ts = []
            for j in range(2):
                wv_t = wqp.tile([128, 2 * NH], F16, tag="wq", name=f"wvup{j}")
                nc.sync.dma_start(out=wv_t[:], in_=wvup[j])
                wvup_ts.append(wv_t)
            vnew = sb.tile([8, NH], F32, tag="vnew")
            for n in range(4):
                ps_v = psbank.tile([8, 512], F32, tag="bank")
                for cc in range(4):
                    nc.tensor.matmul(
                        ps_v[:8, :],
                        ckvT16[:, cc * 8:(cc + 1) * 8],
                        wvup_ts[cc // 2][:, (cc % 2) * NH + n * 512:
                                         (cc % 2) * NH + (n + 1) * 512],
                        start=(cc == 0), stop=(cc == 3),
                    )
                nc.vector.tensor_copy(vnew[:, n * 512:(n + 1) * 512], ps_v[:8, :])
            ps_vT = pstr.tile([128, 128], F32, tag="tr")
            for h in range(HP):
                nc.tensor.transpose(
                    ps_vT[0:128, h * 8:(h + 1) * 8],
                    vnew[:, h * D:(h + 1) * D],
                    id8,
                )
            vnewT = sb.tile([128, 128], F32, tag="vnewT")
            nc.vector.tensor_copy(vnewT[:], ps_vT[:])

            # ---------------- phase B: attn rows = probs @ V ----------------
            attn = sb.tile([128, 128], F32, tag="attn")
            for s in range(16):
                v_t = vp.tile([128, 4096], F16, tag="v")
                nc.sync.dma_start(out=v_t[:], in_=v[s])
                pa = 32 * (s // 4)
                for gg in range(2):
                    g = 2 * s + gg
                    ps_a = psbank.tile([128, 512], F32, tag="bank")
                    for cc in range(4):
                        nc.tensor.matmul(
                            ps_a[:],
                            probsT[:, cc * 128:(cc + 1) * 128],
                            v_t[:, gg * 2048 + cc * 512:gg * 2048 + (cc + 1) * 512],
                            start=(cc == 0), stop=(cc == 3),
                        )
                    for u in range(4):
                        hb = 4 * g + u
                        if F_PPRED:
                            nc.vector.copy_predicated(
                                attn[pa:pa + 32, :],
                                identu8[pa:pa + 32, hb:hb + 1].broadcast_to((32, 128)),
                                ps_a[pa:pa + 32, u * 128:(u + 1) * 128],
                            )
                        else:
                            nc.vector.copy_predicated(
                                attn[:],
                                identu8[:, hb:hb + 1].broadcast_to((128, 128)),
                                ps_a[:, u * 128:(u + 1) * 128],
                            )

            # attnT = attn^T + v_new^T  (f16 for o_proj)
            ps_aT = pstr.tile([128, 128], F32, tag="tr")
            nc.tensor.transpose(ps_aT[:], attn[:], ident[:])
            attnT = sb.tile([128, 128], F16, tag="attnT")
            nc.vector.tensor_add(attnT[:], ps_aT[:], vnewT[:])

            # ---------------- phase C: o_part = attn^T @ Wo_c, RS ----------
            if F_CHUNKRS:
                o_rss = []
                for r, (n0, n1) in enumerate(ROUNDS):
                    nn = n1 - n0
                    ps_os = [
                        psbank.tile([8, 512], F32, tag="bank", name=f"ps_o{r}_{i}")
                        for i in range(nn)
                    ]
                    for hp in range(8):
                        wo_t = vp.tile([128, 2 * 6 * 512], F16, tag="v")
                        nc.sync.dma_start(out=wo_t[:, 0:2 * nn * 512], in_=wos[r][hp])
                        for i2 in range(2):
                            h = 2 * hp + i2
                            for i in range(nn):
                                nc.tensor.matmul(
                                    ps_os[i][:8, :],
                                    attnT[:, h * 8:(h + 1) * 8],
                                    wo_t[:, (i2 * nn + i) * 512:(i2 * nn + i + 1) * 512],
                                    start=(h == 0), stop=(h == 15),
                                )
                    ostage = stg.tile([8, 6 * 512], F32, tag="ostage")
                    for i in range(nn):
                        nc.vector.tensor_copy(
                            ostage[:, i * 512:(i + 1) * 512], ps_os[i][:8, :]
                        )
                    o_bounce = dram.tile([B, nn * 512], F32, tag=f"ob{r}")
                    nc.sync.dma_start(out=o_bounce[:], in_=ostage[:, 0:nn * 512])
                    o_rs = dram.tile([1, nn * 512], F32, tag=f"ors{r}")
                    nc.gpsimd.collective_compute(
                        "ReduceScatter",
                        mybir.AluOpType.add,
                        replica_groups=rg,
                        ins=[o_bounce.opt()],
                        outs=[o_rs.opt()],
                    )
                    o_rss.append((o_rs, n0, n1))
                for o_rs, n0, n1 in o_rss:
                    nc.sync.dma_start(out=o[0:1, n0 * 512:n1 * 512], in_=o_rs[:])
            else:
                o_bounce = dram.tile([B, HID], F32, tag="ob")
                for r, (n0, n1) in enumerate(ROUNDS):
                    nn = n1 - n0
                    ps_os = [
                        psbank.tile([8, 512], F32, tag="bank", name=f"ps_o{r}_{i}")
                        for i in range(nn)
                    ]
                    for hp in range(8):
                        wo_t = vp.tile([128, 2 * 6 * 512], F16, tag="v")
                        nc.sync.dma_start(out=wo_t[:, 0:2 * nn * 512], in_=wos[r][hp])
                        for i2 in range(2):
                            h = 2 * hp + i2
                            for i in range(nn):
                                nc.tensor.matmul(
                                    ps_os[i][:8, :],
                                    attnT[:, h * 8:(h + 1) * 8],
                                    wo_t[:, (i2 * nn + i) * 512:(i2 * nn + i + 1) * 512],
                                    start=(h == 0), stop=(h == 15),
                                )
                    ostage = stg.tile([8, 6 * 512], F32, tag="ostage")
                    for i in range(nn):
                        nc.vector.tensor_copy(
                            ostage[:, i * 512:(i + 1) * 512], ps_os[i][:8, :]
                        )
                    nc.sync.dma_start(
                        out=o_bounce[:, n0 * 512:n1 * 512],
                        in_=ostage[:, 0:nn * 512],
                    )
                o_rs = dram.tile([1, HID], F32, tag="ors")
                nc.gpsimd.collective_compute(
                    "ReduceScatter",
                    mybir.AluOpType.add,
                    replica_groups=rg,
                    ins=[o_bounce.opt()],
                    outs=[o_rs.opt()],
                )
                nc.sync.dma_start(out=o[:], in_=o_rs[:])

    nc.compile()
    return nc


_NC_CACHE = None


def _get_nc():
    global _NC_CACHE
    if _NC_CACHE is None:
        _NC_CACHE = build_nc()
    return _NC_CACHE


def make_in_maps(x, k_cache, v_cache, Wq_down, Wq_up, Wkv_down, Wv_up, Wo):
    f16 = np.float16
    x2 = np.asarray(x, dtype=np.float32).reshape(B, HID).T  # [7168, 8]
    xt_t = np.ascontiguousarray(
        x2.reshape(56, 128, B).transpose(1, 0, 2).reshape(128, 56 * B).astype(f16)
    )
    Wq_down = np.asarray(Wq_down, dtype=np.float32)
    Wq_up = np.asarray(Wq_up, dtype=np.float32)
    Wkv_down = np.asarray(Wkv_down, dtype=np.float32)
    Wv_up = np.asarray(Wv_up, dtype=np.float32)
    Wo = np.asarray(Wo, dtype=np.float32)
    k_cache = np.asarray(k_cache, dtype=np.float32)
    v_cache = np.asarray(v_cache, dtype=np.float32)

    in_maps = []
    for c in range(NC_):
        hs = slice(c * HP, (c + 1) * HP)
        wqd_c = (
            Wq_down[:, c * QLP:(c + 1) * QLP]
            .reshape(2, 28, 128, QLP).transpose(0, 2, 1, 3)
            .reshape(2, 128, 28 * QLP).astype(f16)
        )
        wkvd_c = (
            Wkv_down[c * KVRP:(c + 1) * KVRP, :]
            .reshape(7, 128, KVL).transpose(1, 0, 2).reshape(128, 7 * KVL)
            .astype(f16)
        )
        xkv_c = np.ascontiguousarray(
            x2.reshape(56, 128, B)[7 * c:7 * c + 7]
            .transpose(1, 0, 2).reshape(128, 7 * B).astype(f16)
        )
        wq_shard = Wq_up[:, c * NH:(c + 1) * NH]
        # pad rows to 256 per rank (rows r*256+192..r*256+255 zero) so rhs
        # partition offsets match the gathered cq layout
        wq_pad = np.zeros((2048, NH), np.float32)
        for r in range(8):
            wq_pad[r * 256:r * 256 + QLP] = wq_shard[r * QLP:(r + 1) * QLP]
        wq_c = np.stack([
            wq_pad[:, n * 512:(n + 1) * 512]
            .reshape(16, 128, 512).transpose(1, 0, 2).reshape(128, 16 * 512)
            for n in range(4)
        ]).astype(f16)
        # wvup: [512, 2048] -> 2 chunks of cc-pairs [128, 2*2048]
        wvup_c = (
            Wv_up[:, c * NH:(c + 1) * NH]
            .reshape(2, 2, 128, NH).transpose(0, 2, 1, 3).reshape(2, 128, 2 * NH)
            .astype(f16)
        )
        kt_c = (
            k_cache[:, hs]
            .transpose(1, 0, 3, 2)          # (16, 8, 128, 512) [h, b, d, k]
            .reshape(32, 4, 128, 512)       # [g, t, d, k]
            .transpose(0, 2, 1, 3)          # [g, d, t, k]
            .reshape(16, 2, 128, 2048)      # [s, g2, d, tk]
            .transpose(0, 2, 1, 3)
            .reshape(16, 128, 4096)
            .astype(f16)
        )
        v_c = (
            v_cache[:, hs]
            .transpose(1, 0, 2, 3)          # (16, 8, 512, 128) [h, b, l, d]
            .reshape(32, 4, 4, 128, 128)    # [g, t, c, k, d]
            .transpose(0, 3, 2, 1, 4)       # [g, k, c, t, d]
            .reshape(16, 2, 128, 2048)
            .transpose(0, 2, 1, 3)
            .reshape(16, 128, 4096)
            .astype(f16)
        )
        wo_shard = Wo[c * NH:(c + 1) * NH, :]  # [2048, 7168]
        wo_cs = []
        for (n0, n1) in ROUNDS:
            nn = n1 - n0
            wo_r = (
                wo_shard[:, n0 * 512:n1 * 512]
                .reshape(8, 2, 128, nn * 512).transpose(0, 2, 1, 3)
                .reshape(8, 128, 2 * nn * 512).astype(f16)
            )
            wo_cs.append(wo_r)
        in_maps.append(
            {
                "xt": xt_t,
                "xkv": xkv_c,
                "wqd": np.ascontiguousarray(wqd_c),
                "wkvd": np.ascontiguousarray(wkvd_c),
                "wq": np.ascontiguousarray(wq_c),
                "wvup": np.ascontiguousarray(wvup_c),
                "kt": np.ascontiguousarray(kt_c),
                "v": np.ascontiguousarray(v_c),
                "wo0": np.ascontiguousarray(wo_cs[0]),
                "wo1": np.ascontiguousarray(wo_cs[1]),
                "wo2": np.ascontiguousarray(wo_cs[2]),
            }
        )
    return in_maps


def kernel(x, k_cache, v_cache, Wq_down, Wq_up, Wkv_down, Wk_up, Wv_up, Wo, **_):
    in_maps = make_in_maps(
        np.asarray(x), np.asarray(k_cache), np.asarray(v_cache),
        np.asarray(Wq_down), np.asarray(Wq_up),
        np.asarray(Wkv_down), np.asarray(Wv_up), np.asarray(Wo),
    )
    nc = _get_nc()
    res = bass_utils.run_bass_kernel_spmd(nc, in_maps, core_ids=list(range(NC_)))
    out = np.stack([res.results[b]["o"] for b in range(B)], axis=0)  # (8, 1, 7168)
    return np.ascontiguousarray(out, dtype=np.float32)


# revision 24
# speedup vs baseline: 2.0724x; 1.1455x over previous
"""DeepSeek-style MLA decode attention (batch=8, 128 heads, cache 512) on 8 NeuronCores.

Sharding: tensor-parallel over heads (16 heads/core), bf16 on-device (fp16 o_proj).
 - q LoRA down-proj sharded over rank cols (exact per-core cq); Wkv_down sharded
   over input rows (partial c_kv). One small AllGather ships transposed
   cq/ckv-partials to every core early; each core then computes q for its own
   heads with a column shard of Wq_up (no big mid-kernel collective).
 - Phase A uses a masked-q layout: qTm block hb is [128,32] with only column
   hb%32 live, so the 8 score MMs of a super accumulate into one [32,512] PSUM
   tile whose rows are the real score rows -- no per-row extraction; softmax
   EXP reads the PSUM group tile directly.
 - k_cache host-pretransposed to [super, d, keys]; v_cache to [super, k, (c,t,d)].
 - o_proj input rows sharded by head; partial outputs ReduceScattered over the
   batch dim in 3 column chunks (overlapped with o_proj compute); core b
   returns batch b's final row.

Note: the reference's "new token" softmax is over a length-1 axis (== 1.0), so
k_new/Wk_up are dead and the new-token contribution is simply + v_new.
"""

import numpy as np
import ml_dtypes

import concourse.bass as bass
import concourse.mybir as mybir
import concourse.tile as tile
from concourse import bacc
from concourse import bass_utils
from concourse.masks import make_identity

NC_ = 8                      # cores
B = 8                        # batch
H = 128                      # total heads
HP = H // NC_                # 16 heads per core
D = 128                      # head dim
L = 512                      # cache len
HID = 7168
QL = 1536
QLP = QL // NC_              # 192
KVL = 512
KVRP = HID // NC_            # 896 input rows of Wkv_down per core
NH = HP * D                  # 2048 per-core head cols
SCALE = 1.0 / float(np.sqrt(D))
F32 = mybir.dt.float32
F16 = mybir.dt.float16
BF16 = mybir.dt.bfloat16
U8 = mybir.dt.uint8
BF = ml_dtypes.bfloat16

# o_proj column rounds: (start, end) in 512-col chunks of HID
ROUNDS = ((0, 6), (6, 12), (12, 14))


def build_nc():
    nc = bacc.Bacc(
        "TRN2",
        target_bir_lowering=False,
        debug=False,
        enable_asserts=False,
        num_devices=NC_,
    )
    xt = nc.dram_tensor("xt", [128, 56 * B], BF16, kind="ExternalInput").ap()
    xkv = nc.dram_tensor("xkv", [128, 7 * B], BF16, kind="ExternalInput").ap()
    wqd = nc.dram_tensor("wqd", [2, 128, 28 * QLP], BF16, kind="ExternalInput").ap()
    wkvd = nc.dram_tensor("wkvd", [128, 7 * KVL], BF16, kind="ExternalInput").ap()
    wq = nc.dram_tensor("wq", [4, 128, 16 * 512], BF16, kind="ExternalInput").ap()
    wvup = nc.dram_tensor("wvup", [2, 128, 2 * NH], BF16, kind="ExternalInput").ap()
    kt = nc.dram_tensor("kt", [16, 128, 4096], BF16, kind="ExternalInput").ap()
    v = nc.dram_tensor("v", [16, 128, 4096], BF16, kind="ExternalInput").ap()
    wos = [
        nc.dram_tensor(
            f"wo{r}", [8, 128, 2 * (n1 - n0) * 512], F16, kind="ExternalInput"
        ).ap()
        for r, (n0, n1) in enumerate(ROUNDS)
    ]
    o = nc.dram_tensor("o", [1, HID], F32, kind="ExternalOutput").ap()

    rg = [list(range(NC_))]

    with tile.TileContext(nc) as tc:
        with (
            tc.tile_pool(name="const", bufs=1) as constp,
            tc.tile_pool(name="sbuf", bufs=1) as sb,
            tc.tile_pool(name="stage", bufs=1) as stg,
            tc.tile_pool(name="wqdp", bufs=1) as wqdp,
            tc.tile_pool(name="wqp", bufs=2) as wqp,
            tc.tile_pool(name="ktp", bufs=6) as ktp,
            tc.tile_pool(name="vp", bufs=3) as vp,
            tc.tile_pool(name="psbank", bufs=6, space="PSUM") as psbank,
            tc.tile_pool(name="pstr", bufs=2, space="PSUM") as pstr,
            tc.tile_pool(name="dram", bufs=1, space="DRAM") as dram,
        ):
            ident = constp.tile([128, 128], F32)
            make_identity(nc, ident[:])
            id8 = ident[0:8, 0:8]
            # uint8 one-hot columns for CopyPredicated masks (must be int dtype)
            identu8 = constp.tile([128, 128], U8, tag="identu8")
            nc.vector.tensor_copy(identu8[:], ident[:])

            # ---------------- LoRA down: cq = x @ Wq_down_c, ckv partial ------------
            xt_sb = constp.tile([128, 56 * B], BF16, tag="xt")
            nc.sync.dma_start(out=xt_sb[:], in_=xt)
            xkv_sb = constp.tile([128, 7 * B], BF16, tag="xkv")
            nc.sync.dma_start(out=xkv_sb[:], in_=xkv)
            wkvd_sb = constp.tile([128, 7 * KVL], BF16, tag="wkvd")
            nc.sync.dma_start(out=wkvd_sb[:], in_=wkvd)

            ps_cq = psbank.tile([8, 512], F32, tag="bank")
            ps_ckv = psbank.tile([8, 512], F32, tag="bank")
            wqd_ts = []
            for j in range(2):
                wqd_t = wqdp.tile([128, 28 * QLP], BF16, tag="wqd")
                nc.sync.dma_start(out=wqd_t[:], in_=wqd[j])
                wqd_ts.append(wqd_t)
            for i in range(56):
                j, ii = divmod(i, 28)
                nc.tensor.matmul(
                    ps_cq[:8, 0:QLP],
                    xt_sb[:, i * B:(i + 1) * B],
                    wqd_ts[j][:, ii * QLP:(ii + 1) * QLP],
                    start=(i == 0), stop=(i == 55),
                )
            for i in range(7):
                nc.tensor.matmul(
                    ps_ckv[:8, :],
                    xkv_sb[:, i * B:(i + 1) * B],
                    wkvd_sb[:, i * KVL:(i + 1) * KVL],
                    start=(i == 0), stop=(i == 6),
                )
            cdq = sb.tile([8, QLP], F32, tag="cdq")
            nc.vector.tensor_copy(cdq[:], ps_cq[:8, 0:QLP])
            cdkv = sb.tile([8, KVL], F32, tag="cdkv")
            nc.vector.tensor_copy(cdkv[:], ps_ckv[:8, :])

            # transposes -> ag_in staging [128, 6*8] f32
            # x-cols: 0: cq rows 0-127, 1: cq rows 128-191 (parts 0-63, rest zero),
            #         2-5: ckv chunks of 128
            ps_cqT = pstr.tile([128, 16], F32, tag="tr")
            nc.tensor.transpose(ps_cqT[0:128, 0:8], cdq[:, 0:128], id8)
            nc.tensor.transpose(ps_cqT[0:64, 8:16], cdq[:, 128:192], id8)
            ps_ckvT = pstr.tile([128, 32], F32, tag="tr")
            for j in range(4):
                nc.tensor.transpose(
                    ps_ckvT[0:128, j * 8:(j + 1) * 8],
                    cdkv[:, j * 128:(j + 1) * 128],
                    id8,
                )
            ag_in_sb = sb.tile([128, 48], F32, tag="agin")
            nc.vector.memset(ag_in_sb[:, 8:16], 0.0)
            nc.vector.tensor_copy(ag_in_sb[:, 0:8], ps_cqT[:, 0:8])
            nc.vector.tensor_copy(ag_in_sb[0:64, 8:16], ps_cqT[0:64, 8:16])
            nc.vector.tensor_copy(ag_in_sb[:, 16:48], ps_ckvT[:, 0:32])

            ag_in = dram.tile([128, 48], F32, tag="agi")
            nc.sync.dma_start(out=ag_in[:], in_=ag_in_sb[:])
            ag_out = dram.tile([NC_ * 128, 48], F32, tag="ago")
            nc.gpsimd.collective_compute(
                "AllGather",
                mybir.AluOpType.bypass,
                replica_groups=rg,
                ins=[ag_in.opt()],
                outs=[ag_out.opt()],
            )
            # cq_stage [128, (r, x, b)] f32: one clean DMA (1.5KB/partition runs)
            cq_stage = sb.tile([128, 8 * 48], F32, tag="cqstage")
            nc.sync.dma_start(
                out=cq_stage[:].rearrange("p (r c) -> p r c", r=8),
                in_=ag_out[:].rearrange("(r p) c -> p r c", p=128),
            )
            cqmm = sb.tile([128, 8 * 48], BF16, tag="cqmm")
            nc.vector.tensor_copy(cqmm[:], cq_stage[:])

            # ---------------- q_own = cq @ Wq_up_c  (8, 2048) ----------------
            # per rank r: K=128 chunk (cols r*48..) + K=64 chunk (cols r*48+8..)
            qstage = sb.tile([8, NH], F32, tag="qstage")
            for n in range(4):
                wq_t = wqp.tile([128, 16 * 512], BF16, tag="wq")
                nc.sync.dma_start(out=wq_t[:], in_=wq[n])
                ps_q = psbank.tile([8, 512], F32, tag="bank")
                for r in range(8):
                    nc.tensor.matmul(
                        ps_q[:8, :],
                        cqmm[0:128, r * 48:r * 48 + 8],
                        wq_t[0:128, (2 * r) * 512:(2 * r + 1) * 512],
                        start=(r == 0), stop=False,
                    )
                    nc.tensor.matmul(
                        ps_q[:8, :],
                        cqmm[0:64, r * 48 + 8:r * 48 + 16],
                        wq_t[0:64, (2 * r + 1) * 512:(2 * r + 2) * 512],
                        start=False, stop=(r == 7),
                    )
                nc.vector.tensor_copy(qstage[:, n * 512:(n + 1) * 512], ps_q[:8, :])

            # ckv full = sum of the 8 gathered partials -> ckvT16 [128, 4*8] bf16
            ckvT = sb.tile([128, 32], F32, tag="ckvT")
            nc.vector.tensor_copy(ckvT[:], cq_stage[:, 16:48])
            for r in range(1, 8):
                base = r * 48 + 16
                nc.vector.tensor_add(ckvT[:], ckvT[:], cq_stage[:, base:base + 32])
            ckvT16 = sb.tile([128, 32], BF16, tag="ckvT16")
            nc.vector.tensor_copy(ckvT16[:], ckvT[:])

            # qT [128 d, 128 hb] bf16 via 16 transposes
            ps_qT = pstr.tile([128, 128], F32, tag="tr")
            for h in range(HP):
                nc.tensor.transpose(
                    ps_qT[0:128, h * 8:(h + 1) * 8],
                    qstage[:, h * D:(h + 1) * D],
                    id8,
                )
            qT = sb.tile([128, 128], BF16, tag="qT")
            nc.vector.tensor_copy(qT[:], ps_qT[:])

            # masked q: qTm block hb = [128, 32], only column hb%32 live
            qTm = sb.tile([128, 128 * 32], BF16, tag="qTm")
            nc.vector.memset(qTm[:], 0.0)

            # ---------------- phase A: scores over k cache (masked accumulation) ----
            # group a (hb 32a..32a+32) accumulates its 32 score rows into one
            # base-0 [32, 512] PSUM tile; per-group softmax, then [32,128]
            # transposes place the group's columns of probsT by free offset.
            probsT = sb.tile([128, 512], BF16, tag="probsT")
            ps_gs = {}
            for s in range(16):
                a = s // 4
                pa = 32 * a
                kt_t = ktp.tile([128, 4096], BF16, tag="kt")
                nc.sync.dma_start(out=kt_t[:], in_=kt[s])
                if s % 4 == 0:
                    ps_gs[a] = psbank.tile(
                        [32, 512], F32, tag="bank", name=f"ps_g{a}"
                    )
                ps_g = ps_gs[a]
                for u in range(8):
                    hb = 8 * s + u
                    nc.vector.tensor_copy(
                        qTm[:, hb * 32 + (hb % 32):hb * 32 + (hb % 32) + 1],
                        qT[:, hb:hb + 1],
                    )
                    nc.tensor.matmul(
                        ps_g[0:32, :],
                        qTm[:, hb * 32:(hb + 1) * 32],
                        kt_t[:, u * 512:(u + 1) * 512],
                        start=(s % 4 == 0 and u == 0),
                        stop=(s % 4 == 3 and u == 7),
                    )
                if s % 4 == 3:
                    probs_a = sb.tile([32, 512], F32, tag=f"probs{a}")
                    denom_a = sb.tile([32, 1], F32, tag=f"denom{a}")
                    nc.scalar.activation(
                        probs_a[:], ps_g[0:32, :],
                        mybir.ActivationFunctionType.Exp,
                        scale=SCALE, accum_out=denom_a[:],
                    )
                    recip_a = sb.tile([32, 1], F32, tag=f"recip{a}")
                    nc.vector.reciprocal(recip_a[:], denom_a[:])
                    probsn_a = sb.tile([32, 512], F32, tag=f"probsn{a}")
                    nc.vector.tensor_scalar_mul(probsn_a[:], probs_a[:], recip_a[:])
                    for cc in range(4):
                        ps_pT = pstr.tile([128, 32], F32, tag="tr")
                        nc.tensor.transpose(
                            ps_pT[:],
                            probsn_a[0:32, cc * 128:(cc + 1) * 128],
                            ident[0:32, 0:32],
                        )
                        nc.vector.tensor_copy(
                            probsT[:, cc * 128 + pa:cc * 128 + pa + 32], ps_pT[:]
                        )

            # ---------------- v_new = ckv @ Wv_up_c (8, 2048), off critical path ----
            wvup_ts = []
            for j in range(2):
                wv_t = wqp.tile([128, 2 * NH], BF16, tag="wq", name=f"wvup{j}")
                nc.sync.dma_start(out=wv_t[:], in_=wvup[j])
                wvup_ts.append(wv_t)
            vnew = sb.tile([8, NH], F32, tag="vnew")
            for n in range(4):
                ps_v = psbank.tile([8, 512], F32, tag="bank")
                for cc in range(4):
                    nc.tensor.matmul(
                        ps_v[:8, :],
                        ckvT16[:, cc * 8:(cc + 1) * 8],
                        wvup_ts[cc // 2][:, (cc % 2) * NH + n * 512:
                                         (cc % 2) * NH + (n + 1) * 512],
                        start=(cc == 0), stop=(cc == 3),
                    )
                nc.vector.tensor_copy(vnew[:, n * 512:(n + 1) * 512], ps_v[:8, :])
            ps_vT = pstr.tile([128, 128], F32, tag="tr")
            for h in range(HP):
                nc.tensor.transpose(
                    ps_vT[0:128, h * 8:(h + 1) * 8],
                    vnew[:, h * D:(h + 1) * D],
                    id8,
                )
            vnewT = sb.tile([128, 128], F32, tag="vnewT")
            nc.vector.tensor_copy(vnewT[:], ps_vT[:])

            # ---------------- phase B: attn rows = probs @ V ----------------
            attn = sb.tile([128, 128], F32, tag="attn")
            for s in range(16):
                v_t = vp.tile([128, 4096], BF16, tag="v")
                nc.sync.dma_start(out=v_t[:], in_=v[s])
                pa = 32 * (s // 4)
                for gg in range(2):
                    g = 2 * s + gg
                    ps_a = psbank.tile([128, 512], F32, tag="bank")
                    for cc in range(4):
                        nc.tensor.matmul(
                            ps_a[:],
                            probsT[:, cc * 128:(cc + 1) * 128],
                            v_t[:, gg * 2048 + cc * 512:gg * 2048 + (cc + 1) * 512],
                            start=(cc == 0), stop=(cc == 3),
                        )
                    for u in range(4):
                        hb = 4 * g + u
                        nc.vector.copy_predicated(
                            attn[pa:pa + 32, :],
                            identu8[pa:pa + 32, hb:hb + 1].broadcast_to((32, 128)),
                            ps_a[pa:pa + 32, u * 128:(u + 1) * 128],
                        )

            # attnT = attn^T + v_new^T  (f16 for o_proj)
            ps_aT = pstr.tile([128, 128], F32, tag="tr")
            nc.tensor.transpose(ps_aT[:], attn[:], ident[:])
            attnT = sb.tile([128, 128], F16, tag="attnT")
            nc.vector.tensor_add(attnT[:], ps_aT[:], vnewT[:])

            # ---------------- phase C: o_part = attn^T @ Wo_c, chunked RS ----------
            o_rss = []
            for r, (n0, n1) in enumerate(ROUNDS):
                nn = n1 - n0
                ps_os = [
                    psbank.tile([8, 512], F32, tag="bank", name=f"ps_o{r}_{i}")
                    for i in range(nn)
                ]
                for hp in range(8):
                    wo_t = vp.tile([128, 2 * 6 * 512], F16, tag="v")
                    nc.sync.dma_start(out=wo_t[:, 0:2 * nn * 512], in_=wos[r][hp])
                    for i2 in range(2):
                        h = 2 * hp + i2
                        for i in range(nn):
                            nc.tensor.matmul(
                                ps_os[i][:8, :],
                                attnT[:, h * 8:(h + 1) * 8],
                                wo_t[:, (i2 * nn + i) * 512:(i2 * nn + i + 1) * 512],
                                start=(h == 0), stop=(h == 15),
                            )
                ostage = stg.tile([8, 6 * 512], F32, tag="ostage")
                for i in range(nn):
                    nc.vector.tensor_copy(
                        ostage[:, i * 512:(i + 1) * 512], ps_os[i][:8, :]
                    )
                o_bounce = dram.tile([B, nn * 512], F32, tag=f"ob{r}")
                nc.sync.dma_start(out=o_bounce[:], in_=ostage[:, 0:nn * 512])
                o_rs = dram.tile([1, nn * 512], F32, tag=f"ors{r}")
                nc.gpsimd.collective_compute(
                    "ReduceScatter",
                    mybir.AluOpType.add,
                    replica_groups=rg,
                    ins=[o_bounce.opt()],
                    outs=[o_rs.opt()],
                )
                o_rss.append((o_rs, n0, n1))

            for o_rs, n0, n1 in o_rss:
                nc.sync.dma_start(out=o[0:1, n0 * 512:n1 * 512], in_=o_rs[:])

    nc.compile()
    return nc


_NC_CACHE = None


def _get_nc():
    global _NC_CACHE
    if _NC_CACHE is None:
        _NC_CACHE = build_nc()
    return _NC_CACHE


def make_in_maps(x, k_cache, v_cache, Wq_down, Wq_up, Wkv_down, Wv_up, Wo):
    f16 = np.float16
    x2 = np.asarray(x, dtype=np.float32).reshape(B, HID).T  # [7168, 8]
    xt_t = np.ascontiguousarray(
        x2.reshape(56, 128, B).transpose(1, 0, 2).reshape(128, 56 * B).astype(BF)
    )
    Wq_down = np.asarray(Wq_down, dtype=np.float32)
    Wq_up = np.asarray(Wq_up, dtype=np.float32)
    Wkv_down = np.asarray(Wkv_down, dtype=np.float32)
    Wv_up = np.asarray(Wv_up, dtype=np.float32)
    Wo = np.asarray(Wo, dtype=np.float32)
    k_cache = np.asarray(k_cache, dtype=np.float32)
    v_cache = np.asarray(v_cache, dtype=np.float32)

    in_maps = []
    for c in range(NC_):
        hs = slice(c * HP, (c + 1) * HP)
        wqd_c = (
            Wq_down[:, c * QLP:(c + 1) * QLP]
            .reshape(2, 28, 128, QLP).transpose(0, 2, 1, 3)
            .reshape(2, 128, 28 * QLP).astype(BF)
        )
        wkvd_c = (
            Wkv_down[c * KVRP:(c + 1) * KVRP, :]
            .reshape(7, 128, KVL).transpose(1, 0, 2).reshape(128, 7 * KVL)
            .astype(BF)
        )
        xkv_c = np.ascontiguousarray(
            x2.reshape(56, 128, B)[7 * c:7 * c + 7]
            .transpose(1, 0, 2).reshape(128, 7 * B).astype(BF)
        )
        wq_shard = Wq_up[:, c * NH:(c + 1) * NH]
        # pad rows to 256 per rank (rows r*256+192..255 zero) so the K=64
        # chunk sits at partitions 0..64 of its own x-column
        wq_pad = np.zeros((2048, NH), np.float32)
        for r in range(8):
            wq_pad[r * 256:r * 256 + QLP] = wq_shard[r * QLP:(r + 1) * QLP]
        wq_c = np.stack([
            wq_pad[:, n * 512:(n + 1) * 512]
            .reshape(16, 128, 512).transpose(1, 0, 2).reshape(128, 16 * 512)
            for n in range(4)
        ]).astype(BF)
        wvup_c = (
            Wv_up[:, c * NH:(c + 1) * NH]
            .reshape(2, 2, 128, NH).transpose(0, 2, 1, 3).reshape(2, 128, 2 * NH)
            .astype(BF)
        )
        kt_c = (
            k_cache[:, hs]
            .transpose(1, 0, 3, 2)          # (16, 8, 128, 512) [h, b, d, k]
            .reshape(32, 4, 128, 512)       # [g, t, d, k]
            .transpose(0, 2, 1, 3)          # [g, d, t, k]
            .reshape(16, 2, 128, 2048)      # [s, g2, d, tk]
            .transpose(0, 2, 1, 3)
            .reshape(16, 128, 4096)
            .astype(BF)
        )
        v_c = (
            v_cache[:, hs]
            .transpose(1, 0, 2, 3)          # (16, 8, 512, 128) [h, b, l, d]
            .reshape(32, 4, 4, 128, 128)    # [g, t, c, k, d]
            .transpose(0, 3, 2, 1, 4)       # [g, k, c, t, d]
            .reshape(16, 2, 128, 2048)
            .transpose(0, 2, 1, 3)
            .reshape(16, 128, 4096)
            .astype(BF)
        )
        wo_shard = Wo[c * NH:(c + 1) * NH, :]  # [2048, 7168]
        wo_cs = []
        for (n0, n1) in ROUNDS:
            nn = n1 - n0
            wo_r = (
                wo_shard[:, n0 * 512:n1 * 512]
                .reshape(8, 2, 128, nn * 512).transpose(0, 2, 1, 3)
                .reshape(8, 128, 2 * nn * 512).astype(f16)
            )
            wo_cs.append(wo_r)
        in_maps.append(
            {
                "xt": xt_t,
                "xkv": xkv_c,
                "wqd": np.ascontiguousarray(wqd_c),
                "wkvd": np.ascontiguousarray(wkvd_c),
                "wq": np.ascontiguousarray(wq_c),
                "wvup": np.ascontiguousarray(wvup_c),
                "kt": np.ascontiguousarray(kt_c),
                "v": np.ascontiguousarray(v_c),
                "wo0": np.ascontiguousarray(wo_cs[0]),
                "wo1": np.ascontiguousarray(wo_cs[1]),
                "wo2": np.ascontiguousarray(wo_cs[2]),
            }
        )
    return in_maps


def kernel(x, k_cache, v_cache, Wq_down, Wq_up, Wkv_down, Wk_up, Wv_up, Wo, **_):
    in_maps = make_in_maps(
        np.asarray(x), np.asarray(k_cache), np.asarray(v_cache),
        np.asarray(Wq_down), np.asarray(Wq_up),
        np.asarray(Wkv_down), np.asarray(Wv_up), np.asarray(Wo),
    )
    nc = _get_nc()
    res = bass_utils.run_bass_kernel_spmd(nc, in_maps, core_ids=list(range(NC_)))
    out = np.stack([res.results[b]["o"] for b in range(B)], axis=0)  # (8, 1, 7168)
    return np.ascontiguousarray(out, dtype=np.float32)


# revision 25
# speedup vs baseline: 2.1366x; 1.0310x over previous
"""DeepSeek-style MLA decode attention (batch=8, 128 heads, cache 512) on 8 NeuronCores.

Sharding: tensor-parallel over heads (16 heads/core), bf16 on-device (fp16 o_proj).
 - q LoRA down-proj sharded over rank cols (exact per-core cq); Wkv_down sharded
   over input rows (partial c_kv). One small AllGather ships transposed
   cq/ckv-partials to every core early; each core then computes q for its own
   heads with a column shard of Wq_up (no big mid-kernel collective).
 - Phase A uses a masked-q layout: qTm block hb is [128,32] with only column
   hb%32 live, so the 8 score MMs of a super accumulate into one [32,512] PSUM
   tile whose rows are the real score rows -- no per-row extraction; softmax
   EXP reads the PSUM group tile directly.
 - k_cache host-pretransposed to [super, d, keys]; v_cache to [super, k, (c,t,d)].
 - o_proj input rows sharded by head; partial outputs ReduceScattered over the
   batch dim in 3 column chunks (overlapped with o_proj compute); core b
   returns batch b's final row.

Note: the reference's "new token" softmax is over a length-1 axis (== 1.0), so
k_new/Wk_up are dead and the new-token contribution is simply + v_new.
"""

import numpy as np
import ml_dtypes

import concourse.bass as bass
import concourse.mybir as mybir
import concourse.tile as tile
from concourse import bacc
from concourse import bass_utils
from concourse.masks import make_identity

NC_ = 8                      # cores
B = 8                        # batch
H = 128                      # total heads
HP = H // NC_                # 16 heads per core
D = 128                      # head dim
L = 512                      # cache len
HID = 7168
QL = 1536
QLP = QL // NC_              # 192
KVL = 512
KVRP = HID // NC_            # 896 input rows of Wkv_down per core
NH = HP * D                  # 2048 per-core head cols
SCALE = 1.0 / float(np.sqrt(D))
F32 = mybir.dt.float32
F16 = mybir.dt.float16
BF16 = mybir.dt.bfloat16
U8 = mybir.dt.uint8
BF = ml_dtypes.bfloat16

# o_proj column rounds: (start, end) in 512-col chunks of HID
ROUNDS = ((0, 6), (6, 12), (12, 14))


def build_nc():
    nc = bacc.Bacc(
        "TRN2",
        target_bir_lowering=False,
        debug=False,
        enable_asserts=False,
        num_devices=NC_,
    )
    xt = nc.dram_tensor("xt", [128, 56 * B], BF16, kind="ExternalInput").ap()
    xkv = nc.dram_tensor("xkv", [128, 7 * B], BF16, kind="ExternalInput").ap()
    wqd = nc.dram_tensor("wqd", [2, 128, 28 * QLP], BF16, kind="ExternalInput").ap()
    wkvd = nc.dram_tensor("wkvd", [128, 7 * KVL], BF16, kind="ExternalInput").ap()
    wq = nc.dram_tensor("wq", [4, 128, 16 * 512], BF16, kind="ExternalInput").ap()
    wvup = nc.dram_tensor("wvup", [2, 128, 2 * NH], BF16, kind="ExternalInput").ap()
    kt = nc.dram_tensor("kt", [16, 128, 4096], BF16, kind="ExternalInput").ap()
    v = nc.dram_tensor("v", [16, 128, 4096], BF16, kind="ExternalInput").ap()
    wos = [
        nc.dram_tensor(
            f"wo{r}", [8, 128, 2 * (n1 - n0) * 512], F16, kind="ExternalInput"
        ).ap()
        for r, (n0, n1) in enumerate(ROUNDS)
    ]
    o = nc.dram_tensor("o", [1, HID], F32, kind="ExternalOutput").ap()

    rg = [list(range(NC_))]

    with tile.TileContext(nc) as tc:
        with (
            tc.tile_pool(name="const", bufs=1) as constp,
            tc.tile_pool(name="sbuf", bufs=1) as sb,
            tc.tile_pool(name="stage", bufs=1) as stg,
            tc.tile_pool(name="wqdp", bufs=1) as wqdp,
            tc.tile_pool(name="wqp", bufs=2) as wqp,
            tc.tile_pool(name="ktp", bufs=6) as ktp,
            tc.tile_pool(name="vp", bufs=3) as vp,
            tc.tile_pool(name="psbank", bufs=6, space="PSUM") as psbank,
            tc.tile_pool(name="pstr", bufs=2, space="PSUM") as pstr,
            tc.tile_pool(name="dram", bufs=1, space="DRAM") as dram,
        ):
            ident = constp.tile([128, 128], F32)
            make_identity(nc, ident[:])
            id8 = ident[0:8, 0:8]
            # uint8 one-hot columns for CopyPredicated masks (must be int dtype)
            identu8 = constp.tile([128, 128], U8, tag="identu8")
            nc.vector.tensor_copy(identu8[:], ident[:])

            # ---------------- LoRA down: cq = x @ Wq_down_c, ckv partial ------------
            xt_sb = constp.tile([128, 56 * B], BF16, tag="xt")
            nc.sync.dma_start(out=xt_sb[:], in_=xt)
            xkv_sb = constp.tile([128, 7 * B], BF16, tag="xkv")
            nc.sync.dma_start(out=xkv_sb[:], in_=xkv)
            wkvd_sb = constp.tile([128, 7 * KVL], BF16, tag="wkvd")
            nc.sync.dma_start(out=wkvd_sb[:], in_=wkvd)

            ps_cq = psbank.tile([8, 512], F32, tag="bank")
            ps_ckv = psbank.tile([8, 512], F32, tag="bank")
            wqd_ts = []
            for j in range(2):
                wqd_t = wqdp.tile([128, 28 * QLP], BF16, tag="wqd")
                nc.sync.dma_start(out=wqd_t[:], in_=wqd[j])
                wqd_ts.append(wqd_t)
            for i in range(56):
                j, ii = divmod(i, 28)
                nc.tensor.matmul(
                    ps_cq[:8, 0:QLP],
                    xt_sb[:, i * B:(i + 1) * B],
                    wqd_ts[j][:, ii * QLP:(ii + 1) * QLP],
                    start=(i == 0), stop=(i == 55),
                )
            for i in range(7):
                nc.tensor.matmul(
                    ps_ckv[:8, :],
                    xkv_sb[:, i * B:(i + 1) * B],
                    wkvd_sb[:, i * KVL:(i + 1) * KVL],
                    start=(i == 0), stop=(i == 6),
                )
            cdq = sb.tile([8, QLP], F32, tag="cdq")
            nc.vector.tensor_copy(cdq[:], ps_cq[:8, 0:QLP])
            cdkv = sb.tile([8, KVL], F32, tag="cdkv")
            nc.vector.tensor_copy(cdkv[:], ps_ckv[:8, :])

            # transposes -> ag_in staging [128, 6*8] f32
            # x-cols: 0: cq rows 0-127, 1: cq rows 128-191 (parts 0-63, rest zero),
            #         2-5: ckv chunks of 128
            ps_cqT = pstr.tile([128, 16], F32, tag="tr")
            nc.tensor.transpose(ps_cqT[0:128, 0:8], cdq[:, 0:128], id8)
            nc.tensor.transpose(ps_cqT[0:64, 8:16], cdq[:, 128:192], id8)
            ps_ckvT = pstr.tile([128, 32], F32, tag="tr")
            for j in range(4):
                nc.tensor.transpose(
                    ps_ckvT[0:128, j * 8:(j + 1) * 8],
                    cdkv[:, j * 128:(j + 1) * 128],
                    id8,
                )
            ag_in_sb = sb.tile([128, 48], F32, tag="agin")
            nc.vector.memset(ag_in_sb[:, 8:16], 0.0)
            nc.vector.tensor_copy(ag_in_sb[:, 0:8], ps_cqT[:, 0:8])
            nc.vector.tensor_copy(ag_in_sb[0:64, 8:16], ps_cqT[0:64, 8:16])
            nc.vector.tensor_copy(ag_in_sb[:, 16:48], ps_ckvT[:, 0:32])

            ag_in = dram.tile([128, 48], F32, tag="agi")
            nc.sync.dma_start(out=ag_in[:], in_=ag_in_sb[:])
            ag_out = dram.tile([NC_ * 128, 48], F32, tag="ago")
            nc.gpsimd.collective_compute(
                "AllGather",
                mybir.AluOpType.bypass,
                replica_groups=rg,
                ins=[ag_in.opt()],
                outs=[ag_out.opt()],
            )
            # cq_stage [128, (r, x, b)] f32: one clean DMA (1.5KB/partition runs)
            cq_stage = sb.tile([128, 8 * 48], F32, tag="cqstage")
            nc.sync.dma_start(
                out=cq_stage[:].rearrange("p (r c) -> p r c", r=8),
                in_=ag_out[:].rearrange("(r p) c -> p r c", p=128),
            )
            cqmm = sb.tile([128, 8 * 48], BF16, tag="cqmm")
            nc.vector.tensor_copy(cqmm[:], cq_stage[:])

            # ---------------- q_own = cq @ Wq_up_c  (8, 2048) ----------------
            # per rank r: K=128 chunk (cols r*48..) + K=64 chunk (cols r*48+8..)
            qstage = sb.tile([8, NH], F32, tag="qstage")
            for n in range(4):
                wq_t = wqp.tile([128, 16 * 512], BF16, tag="wq")
                nc.sync.dma_start(out=wq_t[:], in_=wq[n])
                ps_q = psbank.tile([8, 512], F32, tag="bank")
                for r in range(8):
                    nc.tensor.matmul(
                        ps_q[:8, :],
                        cqmm[0:128, r * 48:r * 48 + 8],
                        wq_t[0:128, (2 * r) * 512:(2 * r + 1) * 512],
                        start=(r == 0), stop=False,
                    )
                    nc.tensor.matmul(
                        ps_q[:8, :],
                        cqmm[0:64, r * 48 + 8:r * 48 + 16],
                        wq_t[0:64, (2 * r + 1) * 512:(2 * r + 2) * 512],
                        start=False, stop=(r == 7),
                    )
                nc.vector.tensor_copy(qstage[:, n * 512:(n + 1) * 512], ps_q[:8, :])

            # ckv full = sum of the 8 gathered partials -> ckvT16 [128, 4*8] bf16
            ckvT = sb.tile([128, 32], F32, tag="ckvT")
            nc.vector.tensor_copy(ckvT[:], cq_stage[:, 16:48])
            for r in range(1, 8):
                base = r * 48 + 16
                nc.vector.tensor_add(ckvT[:], ckvT[:], cq_stage[:, base:base + 32])
            ckvT16 = sb.tile([128, 32], BF16, tag="ckvT16")
            nc.vector.tensor_copy(ckvT16[:], ckvT[:])

            # qT [128 d, 128 hb] bf16 via 16 transposes
            ps_qT = pstr.tile([128, 128], F32, tag="tr")
            for h in range(HP):
                nc.tensor.transpose(
                    ps_qT[0:128, h * 8:(h + 1) * 8],
                    qstage[:, h * D:(h + 1) * D],
                    id8,
                )
            qT = sb.tile([128, 128], BF16, tag="qT")
            for n in range(4):
                nc.vector.tensor_copy(
                    qT[:, 32 * n:32 * n + 32], ps_qT[:, 32 * n:32 * n + 32]
                )

            # masked q: qTm block hb = [128, 32], only column hb%32 live
            qTm = sb.tile([128, 128 * 32], BF16, tag="qTm")
            nc.vector.memset(qTm[:], 0.0)

            # ---------------- phase A: scores over k cache (masked accumulation) ----
            # group a (hb 32a..32a+32) accumulates its 32 score rows into one
            # base-0 [32, 512] PSUM tile; per-group softmax, then [32,128]
            # transposes place the group's columns of probsT by free offset.
            probsT = sb.tile([128, 512], BF16, tag="probsT")
            ps_gs = {}
            for s in range(16):
                a = s // 4
                pa = 32 * a
                kt_t = ktp.tile([128, 4096], BF16, tag="kt")
                nc.sync.dma_start(out=kt_t[:], in_=kt[s])
                if s % 4 == 0:
                    ps_gs[a] = psbank.tile(
                        [32, 512], F32, tag="bank", name=f"ps_g{a}"
                    )
                ps_g = ps_gs[a]
                for u in range(8):
                    hb = 8 * s + u
                    nc.vector.tensor_copy(
                        qTm[:, hb * 32 + (hb % 32):hb * 32 + (hb % 32) + 1],
                        qT[:, hb:hb + 1],
                    )
                    nc.tensor.matmul(
                        ps_g[0:32, :],
                        qTm[:, hb * 32:(hb + 1) * 32],
                        kt_t[:, u * 512:(u + 1) * 512],
                        start=(s % 4 == 0 and u == 0),
                        stop=(s % 4 == 3 and u == 7),
                    )
                if s % 4 == 3:
                    probs_a = sb.tile([32, 512], F32, tag=f"probs{a}")
                    denom_a = sb.tile([32, 1], F32, tag=f"denom{a}")
                    nc.scalar.activation(
                        probs_a[:], ps_g[0:32, :],
                        mybir.ActivationFunctionType.Exp,
                        scale=SCALE, accum_out=denom_a[:],
                    )
                    recip_a = sb.tile([32, 1], F32, tag=f"recip{a}")
                    nc.vector.reciprocal(recip_a[:], denom_a[:])
                    probsn_a = sb.tile([32, 512], F32, tag=f"probsn{a}")
                    nc.vector.tensor_scalar_mul(probsn_a[:], probs_a[:], recip_a[:])
                    for cc in range(4):
                        ps_pT = pstr.tile([128, 32], F32, tag="tr")
                        nc.tensor.transpose(
                            ps_pT[:],
                            probsn_a[0:32, cc * 128:(cc + 1) * 128],
                            ident[0:32, 0:32],
                        )
                        nc.vector.tensor_copy(
                            probsT[:, cc * 128 + pa:cc * 128 + pa + 32], ps_pT[:]
                        )

            # ---------------- v_new = ckv @ Wv_up_c (8, 2048), off critical path ----
            wvup_ts = []
            for j in range(2):
                wv_t = wqp.tile([128, 2 * NH], BF16, tag="wq", name=f"wvup{j}")
                nc.sync.dma_start(out=wv_t[:], in_=wvup[j])
                wvup_ts.append(wv_t)
            vnew = sb.tile([8, NH], F32, tag="vnew")
            for n in range(4):
                ps_v = psbank.tile([8, 512], F32, tag="bank")
                for cc in range(4):
                    nc.tensor.matmul(
                        ps_v[:8, :],
                        ckvT16[:, cc * 8:(cc + 1) * 8],
                        wvup_ts[cc // 2][:, (cc % 2) * NH + n * 512:
                                         (cc % 2) * NH + (n + 1) * 512],
                        start=(cc == 0), stop=(cc == 3),
                    )
                nc.vector.tensor_copy(vnew[:, n * 512:(n + 1) * 512], ps_v[:8, :])
            ps_vT = pstr.tile([128, 128], F32, tag="tr")
            for h in range(HP):
                nc.tensor.transpose(
                    ps_vT[0:128, h * 8:(h + 1) * 8],
                    vnew[:, h * D:(h + 1) * D],
                    id8,
                )
            vnewT = sb.tile([128, 128], F32, tag="vnewT")
            nc.vector.tensor_copy(vnewT[:], ps_vT[:])

            # ---------------- phase B: attn rows = probs @ V ----------------
            attn = sb.tile([128, 128], F32, tag="attn")
            for s in range(16):
                v_t = vp.tile([128, 4096], BF16, tag="v")
                nc.sync.dma_start(out=v_t[:], in_=v[s])
                pa = 32 * (s // 4)
                for gg in range(2):
                    g = 2 * s + gg
                    ps_a = psbank.tile([128, 512], F32, tag="bank")
                    for cc in range(4):
                        nc.tensor.matmul(
                            ps_a[:],
                            probsT[:, cc * 128:(cc + 1) * 128],
                            v_t[:, gg * 2048 + cc * 512:gg * 2048 + (cc + 1) * 512],
                            start=(cc == 0), stop=(cc == 3),
                        )
                    for u in range(4):
                        hb = 4 * g + u
                        nc.vector.copy_predicated(
                            attn[pa:pa + 32, :],
                            identu8[pa:pa + 32, hb:hb + 1].broadcast_to((32, 128)),
                            ps_a[pa:pa + 32, u * 128:(u + 1) * 128],
                        )

            # attnT = attn^T + v_new^T  (f16 for o_proj)
            ps_aT = pstr.tile([128, 128], F32, tag="tr")
            nc.tensor.transpose(ps_aT[:], attn[:], ident[:])
            attnT = sb.tile([128, 128], F16, tag="attnT")
            nc.vector.tensor_add(attnT[:], ps_aT[:], vnewT[:])

            # ---------------- phase C: o_part = attn^T @ Wo_c, chunked RS ----------
            o_rss = []
            for r, (n0, n1) in enumerate(ROUNDS):
                nn = n1 - n0
                ps_os = [
                    psbank.tile([8, 512], F32, tag="bank", name=f"ps_o{r}_{i}")
                    for i in range(nn)
                ]
                for hp in range(8):
                    wo_t = vp.tile([128, 2 * 6 * 512], F16, tag="v")
                    nc.sync.dma_start(out=wo_t[:, 0:2 * nn * 512], in_=wos[r][hp])
                    for i2 in range(2):
                        h = 2 * hp + i2
                        for i in range(nn):
                            nc.tensor.matmul(
                                ps_os[i][:8, :],
                                attnT[:, h * 8:(h + 1) * 8],
                                wo_t[:, (i2 * nn + i) * 512:(i2 * nn + i + 1) * 512],
                                start=(h == 0), stop=(h == 15),
                            )
                ostage = stg.tile([8, 6 * 512], F32, tag="ostage")
                for i in range(nn):
                    nc.vector.tensor_copy(
                        ostage[:, i * 512:(i + 1) * 512], ps_os[i][:8, :]
                    )
                o_bounce = dram.tile([B, nn * 512], F32, tag=f"ob{r}")
                nc.sync.dma_start(out=o_bounce[:], in_=ostage[:, 0:nn * 512])
                o_rs = dram.tile([1, nn * 512], F32, tag=f"ors{r}")
                nc.gpsimd.collective_compute(
                    "ReduceScatter",
                    mybir.AluOpType.add,
                    replica_groups=rg,
                    ins=[o_bounce.opt()],
                    outs=[o_rs.opt()],
                )
                o_rss.append((o_rs, n0, n1))

            for o_rs, n0, n1 in o_rss:
                nc.sync.dma_start(out=o[0:1, n0 * 512:n1 * 512], in_=o_rs[:])

    nc.compile()
    return nc


_NC_CACHE = None


def _get_nc():
    global _NC_CACHE
    if _NC_CACHE is None:
        _NC_CACHE = build_nc()
    return _NC_CACHE


def make_in_maps(x, k_cache, v_cache, Wq_down, Wq_up, Wkv_down, Wv_up, Wo):
    f16 = np.float16
    x2 = np.asarray(x, dtype=np.float32).reshape(B, HID).T  # [7168, 8]
    xt_t = np.ascontiguousarray(
        x2.reshape(56, 128, B).transpose(1, 0, 2).reshape(128, 56 * B).astype(BF)
    )
    Wq_down = np.asarray(Wq_down, dtype=np.float32)
    Wq_up = np.asarray(Wq_up, dtype=np.float32)
    Wkv_down = np.asarray(Wkv_down, dtype=np.float32)
    Wv_up = np.asarray(Wv_up, dtype=np.float32)
    Wo = np.asarray(Wo, dtype=np.float32)
    k_cache = np.asarray(k_cache, dtype=np.float32)
    v_cache = np.asarray(v_cache, dtype=np.float32)

    in_maps = []
    for c in range(NC_):
        hs = slice(c * HP, (c + 1) * HP)
        wqd_c = (
            Wq_down[:, c * QLP:(c + 1) * QLP]
            .reshape(2, 28, 128, QLP).transpose(0, 2, 1, 3)
            .reshape(2, 128, 28 * QLP).astype(BF)
        )
        wkvd_c = (
            Wkv_down[c * KVRP:(c + 1) * KVRP, :]
            .reshape(7, 128, KVL).transpose(1, 0, 2).reshape(128, 7 * KVL)
            .astype(BF)
        )
        xkv_c = np.ascontiguousarray(
            x2.reshape(56, 128, B)[7 * c:7 * c + 7]
            .transpose(1, 0, 2).reshape(128, 7 * B).astype(BF)
        )
        wq_shard = Wq_up[:, c * NH:(c + 1) * NH]
        # pad rows to 256 per rank (rows r*256+192..255 zero) so the K=64
        # chunk sits at partitions 0..64 of its own x-column
        wq_pad = np.zeros((2048, NH), np.float32)
        for r in range(8):
            wq_pad[r * 256:r * 256 + QLP] = wq_shard[r * QLP:(r + 1) * QLP]
        wq_c = np.stack([
            wq_pad[:, n * 512:(n + 1) * 512]
            .reshape(16, 128, 512).transpose(1, 0, 2).reshape(128, 16 * 512)
            for n in range(4)
        ]).astype(BF)
        wvup_c = (
            Wv_up[:, c * NH:(c + 1) * NH]
            .reshape(2, 2, 128, NH).transpose(0, 2, 1, 3).reshape(2, 128, 2 * NH)
            .astype(BF)
        )
        kt_c = (
            k_cache[:, hs]
            .transpose(1, 0, 3, 2)          # (16, 8, 128, 512) [h, b, d, k]
            .reshape(32, 4, 128, 512)       # [g, t, d, k]
            .transpose(0, 2, 1, 3)          # [g, d, t, k]
            .reshape(16, 2, 128, 2048)      # [s, g2, d, tk]
            .transpose(0, 2, 1, 3)
            .reshape(16, 128, 4096)
            .astype(BF)
        )
        v_c = (
            v_cache[:, hs]
            .transpose(1, 0, 2, 3)          # (16, 8, 512, 128) [h, b, l, d]
            .reshape(32, 4, 4, 128, 128)    # [g, t, c, k, d]
            .transpose(0, 3, 2, 1, 4)       # [g, k, c, t, d]
            .reshape(16, 2, 128, 2048)
            .transpose(0, 2, 1, 3)
            .reshape(16, 128, 4096)
            .astype(BF)
        )
        wo_shard = Wo[c * NH:(c + 1) * NH, :]  # [2048, 7168]
        wo_cs = []
        for (n0, n1) in ROUNDS:
            nn = n1 - n0
            wo_r = (
                wo_shard[:, n0 * 512:n1 * 512]
                .reshape(8, 2, 128, nn * 512).transpose(0, 2, 1, 3)
                .reshape(8, 128, 2 * nn * 512).astype(f16)
            )
            wo_cs.append(wo_r)
        in_maps.append(
            {
                "xt": xt_t,
                "xkv": xkv_c,
                "wqd": np.ascontiguousarray(wqd_c),
                "wkvd": np.ascontiguousarray(wkvd_c),
                "wq": np.ascontiguousarray(wq_c),
                "wvup": np.ascontiguousarray(wvup_c),
                "kt": np.ascontiguousarray(kt_c),
                "v": np.ascontiguousarray(v_c),
                "wo0": np.ascontiguousarray(wo_cs[0]),
                "wo1": np.ascontiguousarray(wo_cs[1]),
                "wo2": np.ascontiguousarray(wo_cs[2]),
            }
        )
    return in_maps


def kernel(x, k_cache, v_cache, Wq_down, Wq_up, Wkv_down, Wk_up, Wv_up, Wo, **_):
    in_maps = make_in_maps(
        np.asarray(x), np.asarray(k_cache), np.asarray(v_cache),
        np.asarray(Wq_down), np.asarray(Wq_up),
        np.asarray(Wkv_down), np.asarray(Wv_up), np.asarray(Wo),
    )
    nc = _get_nc()
    res = bass_utils.run_bass_kernel_spmd(nc, in_maps, core_ids=list(range(NC_)))
    out = np.stack([res.results[b]["o"] for b in range(B)], axis=0)  # (8, 1, 7168)
    return np.ascontiguousarray(out, dtype=np.float32)
